# revision 11
# baseline (speedup 1.0000x reference)
"""BitNet GQA attention layer on 8 TRN2 NeuronCores.

Sharding: token-parallel. B*S = 2048 tokens -> 256 per core (core c: batch
c//4, quarter c%4). Weights are split 8-way along the contraction dim for
quantization (exact global absmean via a tiny AllReduce), then the ternary
integer weights are AllGathered in bf16. K/V are AllGathered within each
batch's 4-core group. All BitNet matmuls run as exact integer arithmetic in
bf16 (acts in [-128,127], weights in {-1,0,1}) with fp32 PSUM accumulation;
dequantization scales are applied to the fp32 results.
"""

import sys

sys.path.insert(0, "/opt/trn_rl_repo")

import numpy as np
import ml_dtypes

import concourse.bass as bass
import concourse.mybir as mybir
import concourse.tile as tile
from concourse import bacc
from concourse import bass_utils
from concourse.masks import make_identity

F32 = mybir.dt.float32
BF16 = mybir.dt.bfloat16

B, S, H = 2, 1024, 2048
NH, NKV, HD = 16, 8, 128
NC = 8
T = (B * S) // NC  # 256 tokens per core
TB = T // 128  # 2 token tiles per core
HSL = H // NC  # 256 weight rows per core
EPS = 1e-6
RND = 12582912.0  # 1.5 * 2**23: fp32 add => round-to-nearest-even
INV_SQRT_HD = 1.0 / float(np.sqrt(HD))
KTILES = S // 128  # 8 key tiles per batch
GROUP = 4  # cores per batch

# packed widths in the weight all-gather buffer
OQ, OK, OV, OO = H, NKV * HD, NKV * HD, H  # 2048, 1024, 1024, 2048
WCOLS = OQ + OK + OV + OO  # 6144
WOFF = {"q": 0, "k": OQ, "v": OQ + OK, "o": OQ + OK + OV}
WNUMEL = {"q": OQ * H, "k": OK * H, "v": OV * H, "o": OO * H}
MATS = ("q", "k", "v", "o")

_CACHE = {}


def _build():
    nc = bacc.Bacc("TRN2", target_bir_lowering=False, debug=False, num_devices=NC)

    x_sl = nc.dram_tensor("x_sl", [T, H], F32, kind="ExternalInput")
    cosq = nc.dram_tensor("cosq", [T, HD], F32, kind="ExternalInput")
    sinq = nc.dram_tensor("sinq", [T, HD], F32, kind="ExternalInput")
    cosk = nc.dram_tensor("cosk", [T, HD], F32, kind="ExternalInput")
    sink = nc.dram_tensor("sink", [T, HD], F32, kind="ExternalInput")
    w_sl = {
        "q": nc.dram_tensor("wq_sl", [HSL, OQ], F32, kind="ExternalInput"),
        "k": nc.dram_tensor("wk_sl", [HSL, OK], F32, kind="ExternalInput"),
        "v": nc.dram_tensor("wv_sl", [HSL, OV], F32, kind="ExternalInput"),
        "o": nc.dram_tensor("wo_sl", [HSL, OO], F32, kind="ExternalInput"),
    }
    mask_in = nc.dram_tensor("mask", [128, KTILES, T], BF16, kind="ExternalInput")
    wconst = nc.dram_tensor("wconst", [1, 8], F32, kind="ExternalInput")
    out = nc.dram_tensor("out", [T, H], F32, kind="ExternalOutput")

    with tile.TileContext(nc) as tc:
        _build_body(nc, tc, x_sl, cosq, sinq, cosk, sink, w_sl, mask_in, wconst, out)

    nc.compile()
    return nc


def _build_body(nc, tc, x_sl, cosq, sinq, cosk, sink, w_sl, mask_in, wconst, out):
    sync = nc.sync

    with (
        tc.tile_pool(name="dram", bufs=1, space="DRAM") as dram,
        tc.tile_pool(name="const", bufs=1) as constp,
        tc.tile_pool(name="vecs", bufs=1) as vecs,
        tc.tile_pool(name="persist", bufs=1) as persist,
    ):
        # ---- DRAM bounce buffers for collectives ----
        wag_in = dram.tile([HSL, WCOLS], BF16)
        wint = dram.tile([H, WCOLS], BF16, addr_space="Shared")
        ar_in = dram.tile([1, 8], F32)
        ar_out = dram.tile([1, 8], F32, addr_space="Shared")
        kv_in = dram.tile([128, 4096], BF16)
        kv_out = dram.tile([512, 4096], BF16)

        # ---- constants ----
        ident = constp.tile([128, 128], F32)
        make_identity(nc, ident)
        ones1 = constp.tile([1, 128], F32)
        nc.vector.memset(ones1, 1.0)
        onescol = constp.tile([128, 8], F32)
        nc.vector.memset(onescol, 1.0)
        wconst_sb = constp.tile([1, 8], F32)
        sync.dma_start(wconst_sb, wconst.ap())
        negrnd = constp.tile([128, 1], F32)
        nc.vector.memset(negrnd, -RND)
        epsb = constp.tile([128, 1], F32)
        nc.vector.memset(epsb, EPS)

        # persistent activations
        xqT = persist.tile([128, H // 128, T], BF16)  # [h%128, h//128, t]
        qT = persist.tile([128, NH, T], BF16)  # [d, head, t]
        attn = persist.tile([128, TB, H], F32)  # [t%128, t//128, d]

        # ============ Phase W: weight scales + quantize + allgather ============
        with (
            tc.tile_pool(name="wraw_q", bufs=1) as wraw_q,
            tc.tile_pool(name="wraw_k", bufs=1) as wraw_k,
            tc.tile_pool(name="wraw_v", bufs=1) as wraw_v,
            tc.tile_pool(name="wraw_o", bufs=1) as wraw_o,
            tc.tile_pool(name="wtmp", bufs=2) as wtmp,
            tc.tile_pool(name="wq8", bufs=4) as wq8,
            tc.tile_pool(name="psmall", bufs=2, space="PSUM") as psmall,
        ):
            wpools = {"q": wraw_q, "k": wraw_k, "v": wraw_v, "o": wraw_o}
            ow = {"q": OQ, "k": OK, "v": OV, "o": OO}
            wraw = {}
            wabs0 = vecs.tile([128, 4], F32)
            wabs1 = vecs.tile([128, 4], F32)
            for mi, m in enumerate(MATS):
                for pt, wab in ((0, wabs0), (1, wabs1)):
                    wr = wpools[m].tile([128, ow[m]], F32, name=f"wr_{m}{pt}")
                    sync.dma_start(wr, w_sl[m].ap()[pt * 128 : (pt + 1) * 128, :])
                    wraw[(m, pt)] = wr
                    nc.vector.tensor_reduce(
                        wab[:, mi : mi + 1],
                        wr,
                        axis=mybir.AxisListType.X,
                        op=mybir.AluOpType.add,
                        apply_absolute_value=True,
                    )
            wabsc = vecs.tile([128, 4], F32)
            nc.vector.tensor_add(wabsc, wabs0, wabs1)
            # partition-sum -> row vector [1, 4]
            ps_sr = psmall.tile([1, 4], F32)
            nc.tensor.matmul(
                ps_sr, onescol[:, 0:1], wabsc, start=True, stop=True
            )
            sums8 = vecs.tile([1, 8], F32)
            nc.vector.memset(sums8, 0.0)
            nc.scalar.copy(sums8[:, 0:4], ps_sr)
            sync.dma_start(ar_in, sums8)
            nc.gpsimd.collective_compute(
                "AllReduce",
                mybir.AluOpType.add,
                replica_groups=[list(range(NC))],
                ins=[ar_in.opt()],
                outs=[ar_out.opt()],
            )
            gsr = vecs.tile([1, 8], F32)
            sync.dma_start(gsr, ar_out)
            # s_w = numel / sum(|w|);  1/s_w = sum(|w|) / numel
            r4r = vecs.tile([1, 4], F32)
            nc.vector.reciprocal(r4r, gsr[:, 0:4])
            sw8 = vecs.tile([1, 8], F32)
            nc.vector.tensor_mul(sw8[:, 0:4], r4r, wconst_sb[:, 0:4])
            nc.vector.tensor_mul(sw8[:, 4:8], gsr[:, 0:4], wconst_sb[:, 4:8])
            # broadcast to all 128 partitions: [128, 8]
            pb8 = psmall.tile([128, 8], F32)
            nc.tensor.matmul(pb8, ones1, sw8, start=True, stop=True)
            swb_all = vecs.tile([128, 8], F32)
            nc.scalar.copy(swb_all, pb8)
            swb = {m: swb_all[:, mi : mi + 1] for mi, m in enumerate(MATS)}
            rswb = {m: swb_all[:, 4 + mi : 5 + mi] for mi, m in enumerate(MATS)}

            # quantize: wi = clip(round(w * s_w), -1, 1) as bf16
            for m in MATS:
                for pt in range(2):
                    wr = wraw[(m, pt)]
                    tmp = wtmp.tile([128, ow[m]], F32, tag="wtmp")
                    nc.vector.tensor_scalar(
                        tmp, wr, swb[m], RND, op0=mybir.AluOpType.mult,
                        op1=mybir.AluOpType.add,
                    )
                    nc.vector.tensor_scalar(
                        tmp, tmp, -RND, 1.0, op0=mybir.AluOpType.add,
                        op1=mybir.AluOpType.min,
                    )
                    wi = wq8.tile([128, ow[m]], BF16, tag="wi")
                    nc.vector.tensor_scalar(
                        wi, tmp, -1.0, None, op0=mybir.AluOpType.max
                    )
                    sync.dma_start(
                        wag_in[pt * 128 : (pt + 1) * 128, WOFF[m] : WOFF[m] + ow[m]],
                        wi,
                    )
            nc.gpsimd.collective_compute(
                "AllGather",
                mybir.AluOpType.bypass,
                replica_groups=[list(range(NC))],
                ins=[wag_in.opt()],
                outs=[wint.opt()],
            )

        # ============ Phase X: act_quant(x) + transpose ============
        dqx = []  # per token-tile dequant vec (absmax/127)
        with (
            tc.tile_pool(name="xraw", bufs=2) as xraw,
            tc.tile_pool(name="ptr", bufs=2, space="PSUM") as ptr,
        ):
            for tb in range(TB):
                xs = xraw.tile([128, H], F32, tag="xs")
                sync.dma_start(xs, x_sl.ap()[tb * 128 : (tb + 1) * 128, :])
                axm = vecs.tile([128, 1], F32, name=f"axm{tb}")
                nc.vector.tensor_reduce(
                    axm, xs, axis=mybir.AxisListType.X, op=mybir.AluOpType.max,
                    apply_absolute_value=True,
                )
                rsx = vecs.tile([128, 1], F32, name=f"rsx{tb}")
                nc.vector.reciprocal(rsx, axm)
                sxq = vecs.tile([128, 1], F32, name=f"sxq{tb}")
                nc.vector.tensor_scalar_mul(sxq, rsx, 127.0)
                dq = vecs.tile([128, 1], F32, name=f"dqx{tb}")
                nc.vector.tensor_scalar_mul(dq, axm, 1.0 / 127.0)
                dqx.append(dq)
                xr = xraw.tile([128, H], F32, tag="xr")
                nc.vector.tensor_scalar(
                    xr, xs, sxq, RND, op0=mybir.AluOpType.mult,
                    op1=mybir.AluOpType.add,
                )
                # transpose 128x128 tiles; pack 4 per PSUM bank
                for hg in range(0, H // 128, 4):
                    pt4 = ptr.tile([128, 4, 128], F32, tag="pt4")
                    for i in range(4):
                        hi = hg + i
                        nc.tensor.transpose(
                            pt4[:, i, :], xr[:, hi * 128 : (hi + 1) * 128], ident
                        )
                    nc.scalar.activation(
                        xqT[:, hg : hg + 4, tb * 128 : (tb + 1) * 128],
                        pt4,
                        mybir.ActivationFunctionType.Identity,
                        bias=negrnd,
                    )

        # ============ Phase P: Q/K/V projections ============
        q_sb = persist.tile([128, TB, OQ], F32)
        k_sb = persist.tile([128, TB, OK], F32)
        v_sb = persist.tile([128, TB, OV], BF16)
        dqv = {}
        for m in ("q", "k", "v"):
            for tb in range(TB):
                d = vecs.tile([128, 1], F32, name=f"dqv_{m}{tb}")
                nc.vector.tensor_mul(d, dqx[tb], rswb[m])
                dqv[(m, tb)] = d

        with (
            tc.tile_pool(name="wmm", bufs=4) as wmm,
            tc.tile_pool(name="pproj", bufs=4, space="PSUM") as pproj,
        ):
            for m, dst in (("q", q_sb), ("k", k_sb), ("v", v_sb)):
                o_w = {"q": OQ, "k": OK, "v": OV}[m]
                for oc in range(o_w // 512):
                    wts = []
                    for hi in range(H // 128):
                        wt = wmm.tile([128, 512], BF16, tag="wt")
                        sync.dma_start(
                            wt,
                            wint[
                                hi * 128 : (hi + 1) * 128,
                                WOFF[m] + oc * 512 : WOFF[m] + (oc + 1) * 512,
                            ],
                        )
                        wts.append(wt)
                    for tb in range(TB):
                        pp = pproj.tile([128, 512], F32, tag="pp")
                        for hi in range(H // 128):
                            nc.tensor.matmul(
                                pp,
                                xqT[:, hi, tb * 128 : (tb + 1) * 128],
                                wts[hi],
                                start=(hi == 0),
                                stop=(hi == H // 128 - 1),
                            )
                        nc.vector.tensor_scalar(
                            dst[:, tb, oc * 512 : (oc + 1) * 512],
                            pp,
                            dqv[(m, tb)],
                            None,
                            op0=mybir.AluOpType.mult,
                        )

        # ============ Phase N: QK-RMSNorm + RoPE + transpose ============
        cs = {}
        for nm, t in (("cq", cosq), ("sq", sinq), ("ck", cosk), ("sk", sink)):
            c = constp.tile([128, TB, HD], F32, name=f"cs_{nm}")
            sync.dma_start(
                c, t.ap().rearrange("(a p) d -> p a d", p=128)
            )
            cs[nm] = c

        kT = persist.tile([128, NKV, T], BF16)
        with (
            tc.tile_pool(name="ropes", bufs=4) as ropes,
            tc.tile_pool(name="ptr2", bufs=2, space="PSUM") as ptr2,
        ):
            for src, nheads, cosn, sinn, dstT in (
                (q_sb, NH, "cq", "sq", qT),
                (k_sb, NKV, "ck", "sk", kT),
            ):
                for tb in range(TB):
                    for hg in range(0, nheads, 4):
                        pt4 = ptr2.tile([128, 4, 128], F32, tag="pt4b")
                        for i in range(4):
                            h = hg + i
                            blk = src[:, tb, h * 128 : (h + 1) * 128]
                            sq = ropes.tile([128, 128], F32, tag="sqr")
                            msq = vecs.tile([128, 1], F32, name=f"ms{id(src)}_{tb}_{h}")
                            nc.scalar.activation(
                                sq, blk, mybir.ActivationFunctionType.Square,
                                accum_out=msq,
                            )
                            rms = vecs.tile([128, 1], F32, name=f"rm{id(src)}_{tb}_{h}")
                            nc.scalar.activation(
                                rms, msq, mybir.ActivationFunctionType.Sqrt,
                                scale=1.0 / HD, bias=epsb,
                            )
                            rn = vecs.tile([128, 1], F32, name=f"rn{id(src)}_{tb}_{h}")
                            nc.vector.reciprocal(rn, rms)
                            ra = ropes.tile([128, 128], F32, tag="ra")
                            nc.vector.scalar_tensor_tensor(
                                ra, blk, rn, cs[cosn][:, tb, :],
                                op0=mybir.AluOpType.mult, op1=mybir.AluOpType.mult,
                            )
                            rb = ropes.tile([128, 128], F32, tag="rb")
                            nc.vector.scalar_tensor_tensor(
                                rb[:, 0:64], blk[:, 64:128], rn,
                                cs[sinn][:, tb, 0:64],
                                op0=mybir.AluOpType.mult, op1=mybir.AluOpType.mult,
                            )
                            nc.vector.scalar_tensor_tensor(
                                rb[:, 64:128], blk[:, 0:64], rn,
                                cs[sinn][:, tb, 64:128],
                                op0=mybir.AluOpType.mult, op1=mybir.AluOpType.mult,
                            )
                            rf = ropes.tile([128, 128], F32, tag="rf")
                            nc.vector.tensor_add(rf, ra, rb)
                            nc.tensor.transpose(pt4[:, i, :], rf, ident)
                        nc.scalar.activation(
                            dstT[:, hg : hg + 4, tb * 128 : (tb + 1) * 128],
                            pt4,
                            mybir.ActivationFunctionType.Copy,
                        )

        # ============ Phase G: gather K/V within batch group ============
        sync.dma_start(
            kv_in[:, 0 : NKV * T].rearrange("p (hk t) -> p hk t", hk=NKV), kT
        )
        sync.dma_start(
            kv_in[:, NKV * T : NKV * T + TB * OV].rearrange(
                "p (a d) -> p a d", a=TB
            ),
            v_sb,
        )
        nc.gpsimd.collective_compute(
            "AllGather",
            mybir.AluOpType.bypass,
            replica_groups=[[0, 1, 2, 3], [4, 5, 6, 7]],
            ins=[kv_in.opt()],
            outs=[kv_out.opt()],
        )

        # ============ Phase A: attention ============
        mask_sb = persist.tile([128, KTILES, T], BF16)
        sync.dma_start(mask_sb, mask_in.ap())
        kT_all = persist.tile([128, NKV, KTILES, 128], BF16)
        v_all = persist.tile([128, KTILES, NKV, 130], BF16)
        nc.vector.memset(v_all, 1.0)
        for cb in range(GROUP):
            # kT part: kv_out row = 128*cb + d ; col = hk*256 + a*128 + t
            src_k = kv_out[cb * 128 : (cb + 1) * 128, 0 : NKV * T].rearrange(
                "d (hk t) -> d hk t", hk=NKV
            )
            sync.dma_start(kT_all[:, :, 2 * cb : 2 * cb + 2, :], src_k)
            # v part: row = 128*cb + p ; col = 2048 + a*1024 + hk*128 + d
            src_v = kv_out[
                cb * 128 : (cb + 1) * 128, NKV * T : NKV * T + TB * OV
            ].rearrange("p (a hk d) -> p a hk d", a=TB, hk=NKV)
            sync.dma_start(v_all[:, 2 * cb : 2 * cb + 2, :, 0:128], src_v)

        with (
            tc.tile_pool(name="pscore", bufs=3, space="PSUM") as pscore,
            tc.tile_pool(name="ppv", bufs=3, space="PSUM") as ppv,
            tc.tile_pool(name="pexp", bufs=2) as pexp,
        ):
            for h in range(NH):
                hk = h // 2
                pe = pexp.tile([128, KTILES, T], BF16, tag="pe")
                for j in range(KTILES):
                    st = pscore.tile([128, T], F32, tag="st")
                    nc.tensor.matmul(
                        st, kT_all[:, hk, j, :], qT[:, h, :], start=True, stop=True
                    )
                    nc.scalar.activation(
                        pe[:, j, :], st, mybir.ActivationFunctionType.Exp,
                        scale=INV_SQRT_HD,
                    )
                nc.vector.tensor_mul(pe, pe, mask_sb)
                for tb in range(TB):
                    po = ppv.tile([128, 132], F32, tag="po", padded_shape=[128, 132])
                    for j in range(KTILES):
                        nc.tensor.matmul(
                            po[:, 0:129],
                            pe[:, j, tb * 128 : (tb + 1) * 128],
                            v_all[:, j, hk, 0:129],
                            start=(j == 0),
                            stop=(j == KTILES - 1),
                        )
                    rden = vecs.tile([128, 1], F32, name=f"rden{h}_{tb}")
                    nc.vector.reciprocal(rden, po[:, 128:129])
                    nc.vector.tensor_scalar(
                        attn[:, tb, h * 128 : (h + 1) * 128],
                        po[:, 0:128],
                        rden,
                        None,
                        op0=mybir.AluOpType.mult,
                    )

        # ============ Phase O: act_quant(attn) + o_proj ============
        with (
            tc.tile_pool(name="oq", bufs=2) as oq,
            tc.tile_pool(name="ptr3", bufs=2, space="PSUM") as ptr3,
            tc.tile_pool(name="wmm2", bufs=4) as wmm2,
            tc.tile_pool(name="pproj2", bufs=4, space="PSUM") as pproj2,
            tc.tile_pool(name="osb", bufs=2) as osb,
        ):
            aT = persist.tile([128, H // 128, T], BF16)
            dqo = []
            for tb in range(TB):
                axm = vecs.tile([128, 1], F32, name=f"oaxm{tb}")
                nc.vector.tensor_reduce(
                    axm, attn[:, tb, :], axis=mybir.AxisListType.X,
                    op=mybir.AluOpType.max, apply_absolute_value=True,
                )
                rsx = vecs.tile([128, 1], F32, name=f"orsx{tb}")
                nc.vector.reciprocal(rsx, axm)
                sxq = vecs.tile([128, 1], F32, name=f"osxq{tb}")
                nc.vector.tensor_scalar_mul(sxq, rsx, 127.0)
                dq = vecs.tile([128, 1], F32, name=f"odqx{tb}")
                nc.vector.tensor_scalar_mul(dq, axm, 1.0 / 127.0)
                d2 = vecs.tile([128, 1], F32, name=f"odq2{tb}")
                nc.vector.tensor_mul(d2, dq, rswb["o"])
                dqo.append(d2)
                ar = oq.tile([128, H], F32, tag="ar")
                nc.vector.tensor_scalar(
                    ar, attn[:, tb, :], sxq, RND, op0=mybir.AluOpType.mult,
                    op1=mybir.AluOpType.add,
                )
                for hg in range(0, H // 128, 4):
                    pt4 = ptr3.tile([128, 4, 128], F32, tag="pt4c")
                    for i in range(4):
                        hi = hg + i
                        nc.tensor.transpose(
                            pt4[:, i, :], ar[:, hi * 128 : (hi + 1) * 128], ident
                        )
                    nc.scalar.activation(
                        aT[:, hg : hg + 4, tb * 128 : (tb + 1) * 128],
                        pt4,
                        mybir.ActivationFunctionType.Identity,
                        bias=negrnd,
                    )

            for oc in range(OO // 512):
                wts = []
                for hi in range(H // 128):
                    wt = wmm2.tile([128, 512], BF16, tag="wt2")
                    sync.dma_start(
                        wt,
                        wint[
                            hi * 128 : (hi + 1) * 128,
                            WOFF["o"] + oc * 512 : WOFF["o"] + (oc + 1) * 512,
                        ],
                    )
                    wts.append(wt)
                for tb in range(TB):
                    pp = pproj2.tile([128, 512], F32, tag="pp2")
                    for hi in range(H // 128):
                        nc.tensor.matmul(
                            pp,
                            aT[:, hi, tb * 128 : (tb + 1) * 128],
                            wts[hi],
                            start=(hi == 0),
                            stop=(hi == H // 128 - 1),
                        )
                    ot = osb.tile([128, 512], F32, tag="ot")
                    nc.vector.tensor_scalar(
                        ot, pp, dqo[tb], None, op0=mybir.AluOpType.mult
                    )
                    sync.dma_start(
                        out.ap()[
                            tb * 128 : (tb + 1) * 128, oc * 512 : (oc + 1) * 512
                        ],
                        ot,
                    )


def _host_inputs(x, cos, sin, wq, wk, wv, wo, qn, kn):
    """Build the 8 per-core input maps (pure slicing / layout transforms)."""
    x2 = np.asarray(x, np.float32).reshape(B * S, H)
    cos = np.asarray(cos, np.float32)
    sin = np.asarray(sin, np.float32)
    qn = np.asarray(qn, np.float32)
    kn = np.asarray(kn, np.float32)
    # fold qk-norm weights into rope tables (exact identity when qn=kn=1)
    qn_rot = np.concatenate([qn[HD // 2 :], qn[: HD // 2]])
    kn_rot = np.concatenate([kn[HD // 2 :], kn[: HD // 2]])
    sgn = np.concatenate(
        [-np.ones(HD // 2, np.float32), np.ones(HD // 2, np.float32)]
    )
    cosq_t = cos * qn[None, :]
    sinq_t = sin * (qn_rot * sgn)[None, :]
    cosk_t = cos * kn[None, :]
    sink_t = sin * (kn_rot * sgn)[None, :]

    wt = {
        "q": np.asarray(wq, np.float32).T,  # [H, OQ]
        "k": np.asarray(wk, np.float32).T,
        "v": np.asarray(wv, np.float32).T,
        "o": np.asarray(wo, np.float32).T,  # [H(=in), OO]
    }
    wconst = np.concatenate(
        [
            np.array([WNUMEL[m] for m in MATS], np.float32),
            np.array([1.0 / WNUMEL[m] for m in MATS], np.float32),
        ]
    ).reshape(1, 8)

    in_maps = []
    for c in range(NC):
        qt = c % GROUP
        t0 = qt * T
        # causal mask [p, j, f]: key (128j+p) <= query (t0+f)
        p = np.arange(128)[:, None, None]
        j = np.arange(KTILES)[None, :, None]
        f = np.arange(T)[None, None, :]
        mask = ((128 * j + p) <= (t0 + f)).astype(ml_dtypes.bfloat16)
        m = {
            "x_sl": np.ascontiguousarray(x2[c * T : (c + 1) * T]),
            "cosq": np.ascontiguousarray(cosq_t[t0 : t0 + T]),
            "sinq": np.ascontiguousarray(sinq_t[t0 : t0 + T]),
            "cosk": np.ascontiguousarray(cosk_t[t0 : t0 + T]),
            "sink": np.ascontiguousarray(sink_t[t0 : t0 + T]),
            "wq_sl": np.ascontiguousarray(wt["q"][c * HSL : (c + 1) * HSL]),
            "wk_sl": np.ascontiguousarray(wt["k"][c * HSL : (c + 1) * HSL]),
            "wv_sl": np.ascontiguousarray(wt["v"][c * HSL : (c + 1) * HSL]),
            "wo_sl": np.ascontiguousarray(wt["o"][c * HSL : (c + 1) * HSL]),
            "mask": mask,
            "wconst": wconst,
        }
        in_maps.append(m)
    return in_maps


def kernel(x, cos, sin, wq, wk, wv, wo, qn, kn):
    if "nc" not in _CACHE:
        _CACHE["nc"] = _build()
    nc = _CACHE["nc"]
    in_maps = _host_inputs(x, cos, sin, wq, wk, wv, wo, qn, kn)
    res = bass_utils.run_bass_kernel_spmd(nc, in_maps, core_ids=list(range(NC)))
    outs = [np.asarray(res.results[c]["out"]) for c in range(NC)]
    return np.concatenate(outs, axis=0).reshape(B, S, H).astype(np.float32)


# revision 16
# speedup vs baseline: 1.2484x; 1.2484x over previous
"""BitNet GQA attention layer on 8 TRN2 NeuronCores.

Sharding: token-parallel. B*S = 2048 tokens -> 256 per core (core c: batch
c//4, quarter c%4). Weights are split 8-way along the contraction dim for
quantization (exact global absmean via tiny AllReduces), then the ternary
integer weights are AllGathered in bf16 (three pipelined AGs: k+v first so
K/V projections start early, then q, then o). K/V are AllGathered within
each batch's 4-core group, hidden under Q-projection + Q-rope. All BitNet
matmuls run as exact integer arithmetic in bf16 (acts in [-128,127],
weights in {-1,0,1}) with fp32 PSUM accumulation; dequantization scales are
applied to the fp32 results.
"""

import sys

sys.path.insert(0, "/opt/trn_rl_repo")

import numpy as np
import ml_dtypes

import concourse.bass as bass
import concourse.mybir as mybir
import concourse.tile as tile
from concourse import bacc
from concourse import bass_utils
from concourse.masks import make_identity

F32 = mybir.dt.float32
BF16 = mybir.dt.bfloat16
AX = mybir.AxisListType.X
OP = mybir.AluOpType
AF = mybir.ActivationFunctionType

B, S, H = 2, 1024, 2048
NH, NKV, HD = 16, 8, 128
NC = 8
T = (B * S) // NC  # 256 tokens per core
TB = T // 128  # 2 token tiles per core
HSL = H // NC  # 256 weight rows per core
EPS = 1e-6
RND = 12582912.0  # 1.5 * 2**23: fp32 add => round-to-nearest-even
INV_SQRT_HD = 1.0 / float(np.sqrt(HD))
KTILES = S // 128  # 8 key tiles per batch
GROUP = 4  # cores per batch

OQ, OK, OV, OO = H, NKV * HD, NKV * HD, H  # 2048, 1024, 1024, 2048
OW = {"q": OQ, "k": OK, "v": OV, "o": OO}
WNUMEL = {m: OW[m] * H for m in OW}
HI_N = H // 128  # 16 contraction tiles

_CACHE = {}


def _build():
    nc = bacc.Bacc("TRN2", target_bir_lowering=False, debug=False, num_devices=NC)

    x_sl = nc.dram_tensor("x_sl", [T, H], F32, kind="ExternalInput")
    cosq = nc.dram_tensor("cosq", [T, HD], F32, kind="ExternalInput")
    sinq = nc.dram_tensor("sinq", [T, HD], F32, kind="ExternalInput")
    cosk = nc.dram_tensor("cosk", [T, HD], F32, kind="ExternalInput")
    sink = nc.dram_tensor("sink", [T, HD], F32, kind="ExternalInput")
    w_sl = {
        "q": nc.dram_tensor("wq_sl", [HSL, OQ], F32, kind="ExternalInput"),
        "k": nc.dram_tensor("wk_sl", [HSL, OK], F32, kind="ExternalInput"),
        "v": nc.dram_tensor("wv_sl", [HSL, OV], F32, kind="ExternalInput"),
        "o": nc.dram_tensor("wo_sl", [HSL, OO], F32, kind="ExternalInput"),
    }
    mask_in = nc.dram_tensor("mask", [128, KTILES, T], BF16, kind="ExternalInput")
    # cols 0-3: numel for k,v,q,o ; cols 4-7: 1/numel for k,v,q,o
    wconst = nc.dram_tensor("wconst", [1, 8], F32, kind="ExternalInput")
    out = nc.dram_tensor("out", [T, H], F32, kind="ExternalOutput")

    with tile.TileContext(nc) as tc:
        _build_body(nc, tc, x_sl, cosq, sinq, cosk, sink, w_sl, mask_in, wconst, out)

    nc.compile()
    return nc


def _build_body(nc, tc, x_sl, cosq, sinq, cosk, sink, w_sl, mask_in, wconst, out):
    sync = nc.sync

    with (
        tc.tile_pool(name="dram", bufs=1, space="DRAM") as dram,
        tc.tile_pool(name="const", bufs=1) as constp,
        tc.tile_pool(name="vecs", bufs=1) as vecs,
        tc.tile_pool(name="persist", bufs=1) as persist,
        tc.tile_pool(name="ptrans", bufs=2, space="PSUM") as ptrans,
    ):
        # ---- DRAM bounce buffers for collectives ----
        wag_kv = dram.tile([HSL, OK + OV], BF16)
        wint_kv = dram.tile([H, OK + OV], BF16, addr_space="Shared")
        wag_q = dram.tile([HSL, OQ], BF16)
        wint_q = dram.tile([H, OQ], BF16, addr_space="Shared")
        wag_o = dram.tile([HSL, OO], BF16)
        wint_o = dram.tile([H, OO], BF16, addr_space="Shared")
        ar1_in = dram.tile([1, 8], F32)
        ar1_out = dram.tile([1, 8], F32, addr_space="Shared")
        ar2_in = dram.tile([1, 8], F32)
        ar2_out = dram.tile([1, 8], F32, addr_space="Shared")
        kv_in = dram.tile([128, 4096], BF16)
        kv_out = dram.tile([512, 4096], BF16)

        # ---- constants ----
        ident = constp.tile([128, 128], F32)
        make_identity(nc, ident)
        ones1 = constp.tile([1, 128], F32)
        nc.vector.memset(ones1, 1.0)
        onescol = constp.tile([128, 1], F32)
        nc.vector.memset(onescol, 1.0)
        wconst_sb = constp.tile([1, 8], F32)
        sync.dma_start(wconst_sb, wconst.ap())
        negrnd = constp.tile([128, 1], F32)
        nc.vector.memset(negrnd, -RND)
        epsb = constp.tile([128, 1], F32)
        nc.vector.memset(epsb, EPS)
        cs = {}
        for nm, t in (("cq", cosq), ("sq", sinq), ("ck", cosk), ("sk", sink)):
            c = constp.tile([128, TB, HD], F32, name=f"cs_{nm}")
            sync.dma_start(c, t.ap().rearrange("(a p) d -> p a d", p=128))
            cs[nm] = c
        # persistent activations
        xqT = persist.tile([128, HI_N, T], BF16)  # [h%128, h//128, t]

        # ====== Phase W: weight scales + quantize + pipelined allgathers ======
        def w_scale_group(mats, psmall, ar_in, ar_out, wraws, label):
            """Load slices of `mats`, abs-sum, AllReduce -> [128,4] scale tile
            (cols 0,1 = s_w of mats; cols 2,3 = 1/s_w of mats)."""
            wab = {}
            for m in mats:
                for pt in range(2):
                    wr = wraws[m].tile([128, OW[m]], F32, name=f"wr_{m}{pt}")
                    sync.dma_start(wr, w_sl[m].ap()[pt * 128 : (pt + 1) * 128, :])
                    wab[(m, pt)] = wr
            red0 = vecs.tile([128, 2], F32, name=f"red0_{label}")
            red1 = vecs.tile([128, 2], F32, name=f"red1_{label}")
            for mi, m in enumerate(mats):
                for pt, red in ((0, red0), (1, red1)):
                    nc.vector.tensor_reduce(
                        red[:, mi : mi + 1], wab[(m, pt)], axis=AX, op=OP.add,
                        apply_absolute_value=True,
                    )
            redc = vecs.tile([128, 2], F32, name=f"redc_{label}")
            nc.vector.tensor_add(redc, red0, red1)
            ps = psmall.tile([1, 2], F32, name=f"ps_{label}", tag="psm")
            nc.tensor.matmul(ps, onescol, redc, start=True, stop=True)
            sums = vecs.tile([1, 8], F32, name=f"sums_{label}")
            nc.vector.memset(sums, 0.0)
            nc.scalar.copy(sums[:, 0:2], ps)
            sync.dma_start(ar_in, sums)
            nc.gpsimd.collective_compute(
                "AllReduce", OP.add, replica_groups=[list(range(NC))],
                ins=[ar_in.opt()], outs=[ar_out.opt()],
            )
            g = vecs.tile([1, 8], F32, name=f"g_{label}")
            sync.dma_start(g, ar_out)
            r2 = vecs.tile([1, 2], F32, name=f"r2_{label}")
            nc.vector.reciprocal(r2, g[:, 0:2])
            sw4 = vecs.tile([1, 4], F32, name=f"sw4_{label}")
            ncol = {("k", "v"): (0, 2), ("q", "o"): (2, 4)}[tuple(mats)]
            nc.vector.tensor_mul(sw4[:, 0:2], r2, wconst_sb[:, ncol[0] : ncol[1]])
            nc.vector.tensor_mul(
                sw4[:, 2:4], g[:, 0:2], wconst_sb[:, 4 + ncol[0] : 4 + ncol[1]]
            )
            pb = psmall.tile([128, 4], F32, name=f"pb_{label}", tag="psm")
            nc.tensor.matmul(pb, ones1, sw4, start=True, stop=True)
            sb = vecs.tile([128, 4], F32, name=f"sb_{label}")
            nc.scalar.copy(sb, pb)
            return wab, sb

        def w_quant(wab, m, mi, sb, wtmp, wq8, dst, col0):
            for pt in range(2):
                wr = wab[(m, pt)]
                tmp = wtmp.tile([128, OW[m]], F32, tag="wtmp")
                nc.vector.tensor_scalar(
                    tmp, wr, sb[:, mi : mi + 1], RND, op0=OP.mult, op1=OP.add
                )
                nc.vector.tensor_scalar(
                    tmp, tmp, -RND, 1.0, op0=OP.add, op1=OP.min
                )
                wi = wq8.tile([128, OW[m]], BF16, tag="wi")
                nc.vector.tensor_scalar(wi, tmp, -1.0, None, op0=OP.max)
                sync.dma_start(
                    dst[pt * 128 : (pt + 1) * 128, col0 : col0 + OW[m]], wi
                )

        rswb = {}
        with (
            tc.tile_pool(name="wraw_q", bufs=1) as wraw_q,
            tc.tile_pool(name="wraw_k", bufs=1) as wraw_k,
            tc.tile_pool(name="wraw_v", bufs=1) as wraw_v,
            tc.tile_pool(name="wraw_o", bufs=1) as wraw_o,
            tc.tile_pool(name="wtmp", bufs=2) as wtmp,
            tc.tile_pool(name="wq8", bufs=2) as wq8,
            tc.tile_pool(name="psmall", bufs=2, space="PSUM") as psmall,
        ):
            wraws = {"q": wraw_q, "k": wraw_k, "v": wraw_v, "o": wraw_o}
            wab_kv, sb_kv = w_scale_group(
                ("k", "v"), psmall, ar1_in, ar1_out, wraws, "kv"
            )
            w_quant(wab_kv, "k", 0, sb_kv, wtmp, wq8, wag_kv, 0)
            w_quant(wab_kv, "v", 1, sb_kv, wtmp, wq8, wag_kv, OK)
            nc.gpsimd.collective_compute(
                "AllGather", OP.bypass, replica_groups=[list(range(NC))],
                ins=[wag_kv.opt()], outs=[wint_kv.opt()],
            )
            rswb["k"] = sb_kv[:, 2:3]
            rswb["v"] = sb_kv[:, 3:4]

            wab_qo, sb_qo = w_scale_group(
                ("q", "o"), psmall, ar2_in, ar2_out, wraws, "qo"
            )
            w_quant(wab_qo, "q", 0, sb_qo, wtmp, wq8, wag_q, 0)
            nc.gpsimd.collective_compute(
                "AllGather", OP.bypass, replica_groups=[list(range(NC))],
                ins=[wag_q.opt()], outs=[wint_q.opt()],
            )
            w_quant(wab_qo, "o", 1, sb_qo, wtmp, wq8, wag_o, 0)
            nc.gpsimd.collective_compute(
                "AllGather", OP.bypass, replica_groups=[list(range(NC))],
                ins=[wag_o.opt()], outs=[wint_o.opt()],
            )
            rswb["q"] = sb_qo[:, 2:3]
            rswb["o"] = sb_qo[:, 3:4]

            # ====== Phase X: act_quant(x) + transpose (overlaps Phase W) ======
            dqx = []
            with tc.tile_pool(name="xraw", bufs=2) as xraw:
                for tb in range(TB):
                    xs = xraw.tile([128, H], F32, tag="xs")
                    sync.dma_start(xs, x_sl.ap()[tb * 128 : (tb + 1) * 128, :])
                    axm = vecs.tile([128, 1], F32, name=f"axm{tb}")
                    nc.vector.tensor_reduce(
                        axm, xs, axis=AX, op=OP.max, apply_absolute_value=True
                    )
                    rsx = vecs.tile([128, 1], F32, name=f"rsx{tb}")
                    nc.vector.reciprocal(rsx, axm)
                    sxq = vecs.tile([128, 1], F32, name=f"sxq{tb}")
                    nc.vector.tensor_scalar_mul(sxq, rsx, 127.0)
                    dq = vecs.tile([128, 1], F32, name=f"dqx{tb}")
                    nc.vector.tensor_scalar_mul(dq, axm, 1.0 / 127.0)
                    dqx.append(dq)
                    nc.vector.tensor_scalar(
                        xs, xs, sxq, RND, op0=OP.mult, op1=OP.add
                    )
                    for hg in range(0, HI_N, 4):
                        pt4 = ptrans.tile([128, 4, 128], F32, tag="ptr")
                        for i in range(4):
                            hi = hg + i
                            nc.tensor.transpose(
                                pt4[:, i, :], xs[:, hi * 128 : (hi + 1) * 128], ident
                            )
                        nc.scalar.activation(
                            xqT[:, hg : hg + 4, tb * 128 : (tb + 1) * 128],
                            pt4, AF.Identity, bias=negrnd,
                        )

        # dequant vectors (absmax/127 * 1/s_w)
        dqv = {}
        for m in ("q", "k", "v", "o"):
            for tb in range(TB):
                d = vecs.tile([128, 1], F32, name=f"dqv_{m}{tb}")
                nc.vector.tensor_mul(d, dqx[tb], rswb[m])
                dqv[(m, tb)] = d

        q_sb = persist.tile([128, TB, OQ], F32)
        k_sb = persist.tile([128, TB, OK], F32)
        v_sb = persist.tile([128, TB, OV], BF16)
        qT = persist.tile([128, NH, T], BF16)  # [d, head, t]
        kT = persist.tile([128, NKV, T], BF16)

        def proj(wint_src, col0, o_w, m, dst, wpool, ppool):
            """dst[:, tb, oc*512...] = dequant(xqT.T @ w_int) over o-chunks."""
            src3 = wint_src.rearrange("(hi p) o -> p hi o", p=128)
            for oc in range(o_w // 512):
                wst = wpool.tile([128, HI_N, 512], BF16, tag="wst")
                sync.dma_start(
                    wst, src3[:, :, col0 + oc * 512 : col0 + (oc + 1) * 512]
                )
                for tb in range(TB):
                    pp = ppool.tile([128, 512], F32, tag="pp")
                    for hi in range(HI_N):
                        nc.tensor.matmul(
                            pp,
                            xqT[:, hi, tb * 128 : (tb + 1) * 128],
                            wst[:, hi, :],
                            start=(hi == 0),
                            stop=(hi == HI_N - 1),
                        )
                    nc.vector.tensor_scalar(
                        dst[:, tb, oc * 512 : (oc + 1) * 512],
                        pp, dqv[(m, tb)], None, op0=OP.mult,
                    )

        def rope_batch(src_sb, tb, nh, cosn, sinn, dstT, ropep, label):
            w = nh * 128
            blk = src_sb[:, tb, :]  # [128, w] f32
            sq = ropep.tile([128, w], F32, tag="sq", padded_shape=[128, NH * 128])
            nc.scalar.activation(sq, blk, AF.Square)
            ms = vecs.tile([128, nh], F32, name=f"ms_{label}{tb}")
            nc.vector.tensor_reduce(
                ms, sq.rearrange("p (h d) -> p h d", h=nh), axis=AX, op=OP.add
            )
            rms = vecs.tile([128, nh], F32, name=f"rms_{label}{tb}")
            nc.scalar.activation(rms, ms, AF.Sqrt, scale=1.0 / HD, bias=epsb)
            rn = vecs.tile([128, nh], F32, name=f"rn_{label}{tb}")
            nc.vector.reciprocal(rn, rms)
            rnb = rn.to_broadcast([128, nh, 128])
            blk3 = blk.rearrange("p (h d) -> p h d", h=nh)
            un = ropep.tile([128, nh, 128], F32, tag="un", padded_shape=[128, NH, 128])
            nc.vector.tensor_mul(un, blk3, rnb)
            cosb = (
                cs[cosn][:, tb, :]
                .rearrange("p (one d) -> p one d", one=1)
                .to_broadcast([128, nh, 128])
            )
            sinb = (
                cs[sinn][:, tb, :]
                .rearrange("p (one d) -> p one d", one=1)
                .to_broadcast([128, nh, 128])
            )
            ra = ropep.tile([128, nh, 128], F32, tag="ra", padded_shape=[128, NH, 128])
            nc.vector.tensor_mul(ra, un, cosb)
            rb = ropep.tile([128, nh, 128], F32, tag="rb", padded_shape=[128, NH, 128])
            nc.vector.tensor_mul(rb[:, :, 0:64], un[:, :, 64:128], sinb[:, :, 0:64])
            nc.vector.tensor_mul(rb[:, :, 64:128], un[:, :, 0:64], sinb[:, :, 64:128])
            nc.vector.tensor_add(ra, ra, rb)
            for hg in range(0, nh, 4):
                pt4 = ptrans.tile([128, 4, 128], F32, tag="ptr")
                for i in range(4):
                    nc.tensor.transpose(pt4[:, i, :], ra[:, hg + i, :], ident)
                nc.scalar.activation(
                    dstT[:, hg : hg + 4, tb * 128 : (tb + 1) * 128], pt4, AF.Copy
                )

        # ====== K/V projections + K rope + KV allgather ======
        with (
            tc.tile_pool(name="wmm1", bufs=3) as wmm1,
            tc.tile_pool(name="pproj1", bufs=3, space="PSUM") as pproj1,
            tc.tile_pool(name="ropek", bufs=1) as ropek,
        ):
            proj(wint_kv, 0, OK, "k", k_sb, wmm1, pproj1)
            proj(wint_kv, OK, OV, "v", v_sb, wmm1, pproj1)
            for tb in range(TB):
                rope_batch(k_sb, tb, NKV, "ck", "sk", kT, ropek, "k")

            sync.dma_start(
                kv_in[:, 0 : NKV * T].rearrange("p (hk t) -> p hk t", hk=NKV), kT
            )
            sync.dma_start(
                kv_in[:, NKV * T : NKV * T + TB * OV].rearrange(
                    "p (a d) -> p a d", a=TB
                ),
                v_sb,
            )
            nc.gpsimd.collective_compute(
                "AllGather", OP.bypass,
                replica_groups=[[0, 1, 2, 3], [4, 5, 6, 7]],
                ins=[kv_in.opt()], outs=[kv_out.opt()],
            )

            # ====== Q projection + Q rope (overlap the KV allgather) ======
            proj(wint_q, 0, OQ, "q", q_sb, wmm1, pproj1)
            for tb in range(TB):
                rope_batch(q_sb, tb, NH, "cq", "sq", qT, ropek, "q")

        # ====== gather readback ======
        mask_sb = persist.tile([128, KTILES, T], BF16)
        sync.dma_start(mask_sb, mask_in.ap())
        attn = persist.tile([128, TB, H], F32)  # [t%128, t//128, d]
        kT_all = persist.tile([128, NKV, KTILES, 128], BF16)
        v_all = persist.tile([128, KTILES, NKV, 130], BF16)
        nc.vector.memset(v_all, 1.0)
        for cb in range(GROUP):
            # kT part: kv_out row = 128*cb + d ; col = hk*256 + a*128 + t
            src_k = kv_out[cb * 128 : (cb + 1) * 128, 0 : NKV * T].rearrange(
                "d (hk t) -> d hk t", hk=NKV
            )
            sync.dma_start(kT_all[:, :, 2 * cb : 2 * cb + 2, :], src_k)
            # v part: row = 128*cb + p ; col = 2048 + a*1024 + hk*128 + d
            src_v = kv_out[
                cb * 128 : (cb + 1) * 128, NKV * T : NKV * T + TB * OV
            ].rearrange("p (a hk d) -> p a hk d", a=TB, hk=NKV)
            sync.dma_start(v_all[:, 2 * cb : 2 * cb + 2, :, 0:128], src_v)

        # ====== attention ======
        with (
            tc.tile_pool(name="pscore", bufs=2, space="PSUM") as pscore,
            tc.tile_pool(name="ppv", bufs=2, space="PSUM") as ppv,
            tc.tile_pool(name="pexp", bufs=2) as pexp,
        ):
            for h in range(NH):
                hk = h // 2
                pe = pexp.tile([128, KTILES, T], BF16, tag="pe")
                for j in range(KTILES):
                    st = pscore.tile([128, T], F32, tag="st")
                    nc.tensor.matmul(
                        st, kT_all[:, hk, j, :], qT[:, h, :], start=True, stop=True
                    )
                    nc.scalar.activation(
                        pe[:, j, :], st, AF.Exp, scale=INV_SQRT_HD
                    )
                nc.vector.tensor_mul(pe, pe, mask_sb)
                for tb in range(TB):
                    po = ppv.tile([128, 132], F32, tag="po", padded_shape=[128, 132])
                    for j in range(KTILES):
                        nc.tensor.matmul(
                            po[:, 0:129],
                            pe[:, j, tb * 128 : (tb + 1) * 128],
                            v_all[:, j, hk, 0:129],
                            start=(j == 0),
                            stop=(j == KTILES - 1),
                        )
                    rden = vecs.tile([128, 1], F32, name=f"rden{h}_{tb}")
                    nc.vector.reciprocal(rden, po[:, 128:129])
                    nc.vector.tensor_scalar(
                        attn[:, tb, h * 128 : (h + 1) * 128],
                        po[:, 0:128], rden, None, op0=OP.mult,
                    )

        # ====== act_quant(attn) + o_proj ======
        with (
            tc.tile_pool(name="oq", bufs=2) as oq,
            tc.tile_pool(name="wmm2", bufs=3) as wmm2,
            tc.tile_pool(name="pproj2", bufs=3, space="PSUM") as pproj2,
            tc.tile_pool(name="osb", bufs=2) as osb,
        ):
            aT = persist.tile([128, HI_N, T], BF16)
            dqo = []
            for tb in range(TB):
                axm = vecs.tile([128, 1], F32, name=f"oaxm{tb}")
                nc.vector.tensor_reduce(
                    axm, attn[:, tb, :], axis=AX, op=OP.max,
                    apply_absolute_value=True,
                )
                rsx = vecs.tile([128, 1], F32, name=f"orsx{tb}")
                nc.vector.reciprocal(rsx, axm)
                sxq = vecs.tile([128, 1], F32, name=f"osxq{tb}")
                nc.vector.tensor_scalar_mul(sxq, rsx, 127.0)
                dq = vecs.tile([128, 1], F32, name=f"odqx{tb}")
                nc.vector.tensor_scalar_mul(dq, axm, 1.0 / 127.0)
                d2 = vecs.tile([128, 1], F32, name=f"odq2{tb}")
                nc.vector.tensor_mul(d2, dq, rswb["o"])
                dqo.append(d2)
                ar = oq.tile([128, H], F32, tag="ar")
                nc.vector.tensor_scalar(
                    ar, attn[:, tb, :], sxq, RND, op0=OP.mult, op1=OP.add
                )
                for hg in range(0, HI_N, 4):
                    pt4 = ptrans.tile([128, 4, 128], F32, tag="ptr")
                    for i in range(4):
                        hi = hg + i
                        nc.tensor.transpose(
                            pt4[:, i, :], ar[:, hi * 128 : (hi + 1) * 128], ident
                        )
                    nc.scalar.activation(
                        aT[:, hg : hg + 4, tb * 128 : (tb + 1) * 128],
                        pt4, AF.Identity, bias=negrnd,
                    )

            src3 = wint_o.rearrange("(hi p) o -> p hi o", p=128)
            for oc in range(OO // 512):
                wst = wmm2.tile([128, HI_N, 512], BF16, tag="wst2")
                sync.dma_start(wst, src3[:, :, oc * 512 : (oc + 1) * 512])
                for tb in range(TB):
                    pp = pproj2.tile([128, 512], F32, tag="pp2")
                    for hi in range(HI_N):
                        nc.tensor.matmul(
                            pp,
                            aT[:, hi, tb * 128 : (tb + 1) * 128],
                            wst[:, hi, :],
                            start=(hi == 0),
                            stop=(hi == HI_N - 1),
                        )
                    ot = osb.tile([128, 512], F32, tag="ot")
                    nc.vector.tensor_scalar(ot, pp, dqo[tb], None, op0=OP.mult)
                    sync.dma_start(
                        out.ap()[
                            tb * 128 : (tb + 1) * 128, oc * 512 : (oc + 1) * 512
                        ],
                        ot,
                    )


def _host_inputs(x, cos, sin, wq, wk, wv, wo, qn, kn):
    """Build the 8 per-core input maps (pure slicing / layout transforms)."""
    x2 = np.asarray(x, np.float32).reshape(B * S, H)
    cos = np.asarray(cos, np.float32)
    sin = np.asarray(sin, np.float32)
    qn = np.asarray(qn, np.float32)
    kn = np.asarray(kn, np.float32)
    # fold qk-norm weights into rope tables (exact identity when qn=kn=1)
    qn_rot = np.concatenate([qn[HD // 2 :], qn[: HD // 2]])
    kn_rot = np.concatenate([kn[HD // 2 :], kn[: HD // 2]])
    sgn = np.concatenate(
        [-np.ones(HD // 2, np.float32), np.ones(HD // 2, np.float32)]
    )
    cosq_t = cos * qn[None, :]
    sinq_t = sin * (qn_rot * sgn)[None, :]
    cosk_t = cos * kn[None, :]
    sink_t = sin * (kn_rot * sgn)[None, :]

    wt = {
        "q": np.asarray(wq, np.float32).T,  # [H, OQ]
        "k": np.asarray(wk, np.float32).T,
        "v": np.asarray(wv, np.float32).T,
        "o": np.asarray(wo, np.float32).T,  # [H(=in), OO]
    }
    worder = ("k", "v", "q", "o")
    wconst = np.concatenate(
        [
            np.array([WNUMEL[m] for m in worder], np.float32),
            np.array([1.0 / WNUMEL[m] for m in worder], np.float32),
        ]
    ).reshape(1, 8)

    in_maps = []
    for c in range(NC):
        qt = c % GROUP
        t0 = qt * T
        # causal mask [p, j, f]: key (128j+p) <= query (t0+f)
        p = np.arange(128)[:, None, None]
        j = np.arange(KTILES)[None, :, None]
        f = np.arange(T)[None, None, :]
        mask = ((128 * j + p) <= (t0 + f)).astype(ml_dtypes.bfloat16)
        m = {
            "x_sl": np.ascontiguousarray(x2[c * T : (c + 1) * T]),
            "cosq": np.ascontiguousarray(cosq_t[t0 : t0 + T]),
            "sinq": np.ascontiguousarray(sinq_t[t0 : t0 + T]),
            "cosk": np.ascontiguousarray(cosk_t[t0 : t0 + T]),
            "sink": np.ascontiguousarray(sink_t[t0 : t0 + T]),
            "wq_sl": np.ascontiguousarray(wt["q"][c * HSL : (c + 1) * HSL]),
            "wk_sl": np.ascontiguousarray(wt["k"][c * HSL : (c + 1) * HSL]),
            "wv_sl": np.ascontiguousarray(wt["v"][c * HSL : (c + 1) * HSL]),
            "wo_sl": np.ascontiguousarray(wt["o"][c * HSL : (c + 1) * HSL]),
            "mask": mask,
            "wconst": wconst,
        }
        in_maps.append(m)
    return in_maps


def kernel(x, cos, sin, wq, wk, wv, wo, qn, kn):
    if "nc" not in _CACHE:
        _CACHE["nc"] = _build()
    nc = _CACHE["nc"]
    in_maps = _host_inputs(x, cos, sin, wq, wk, wv, wo, qn, kn)
    res = bass_utils.run_bass_kernel_spmd(nc, in_maps, core_ids=list(range(NC)))
    outs = [np.asarray(res.results[c]["out"]) for c in range(NC)]
    return np.concatenate(outs, axis=0).reshape(B, S, H).astype(np.float32)


# revision 23
# speedup vs baseline: 1.2694x; 1.0168x over previous
"""BitNet GQA attention layer on 8 TRN2 NeuronCores.

Sharding: token-parallel. B*S = 2048 tokens -> 256 per core (core c: batch
c//4, quarter c%4). Weights are split 8-way along the contraction dim for
quantization (exact global absmean via tiny AllReduces), then the ternary
integer weights are AllGathered in bf16 (three pipelined AGs: k+v first so
K/V projections start early, then q, then o). K/V are AllGathered within
each batch's 4-core group, hidden under Q-projection + Q-rope. All BitNet
matmuls run as exact integer arithmetic in bf16 (acts in [-128,127],
weights in {-1,0,1}) with fp32 PSUM accumulation; dequantization scales are
applied to the fp32 results.
"""

import sys

sys.path.insert(0, "/opt/trn_rl_repo")

import numpy as np
import ml_dtypes

import concourse.bass as bass
import concourse.mybir as mybir
import concourse.tile as tile
from concourse import bacc
from concourse import bass_utils
from concourse.masks import make_identity

F32 = mybir.dt.float32
BF16 = mybir.dt.bfloat16
AX = mybir.AxisListType.X
OP = mybir.AluOpType
AF = mybir.ActivationFunctionType

B, S, H = 2, 1024, 2048
NH, NKV, HD = 16, 8, 128
NC = 8
T = (B * S) // NC  # 256 tokens per core
TB = T // 128  # 2 token tiles per core
HSL = H // NC  # 256 weight rows per core
EPS = 1e-6
RND = 12582912.0  # 1.5 * 2**23: fp32 add => round-to-nearest-even
INV_SQRT_HD = 1.0 / float(np.sqrt(HD))
KTILES = S // 128  # 8 key tiles per batch
GROUP = 4  # cores per batch

OQ, OK, OV, OO = H, NKV * HD, NKV * HD, H  # 2048, 1024, 1024, 2048
OW = {"q": OQ, "k": OK, "v": OV, "o": OO}
WNUMEL = {m: OW[m] * H for m in OW}
HI_N = H // 128  # 16 contraction tiles

_CACHE = {}


def _build():
    nc = bacc.Bacc("TRN2", target_bir_lowering=False, debug=False, num_devices=NC)

    x_sl = nc.dram_tensor("x_sl", [T, H], F32, kind="ExternalInput")
    cosq = nc.dram_tensor("cosq", [T, HD], F32, kind="ExternalInput")
    sinq = nc.dram_tensor("sinq", [T, HD], F32, kind="ExternalInput")
    cosk = nc.dram_tensor("cosk", [T, HD], F32, kind="ExternalInput")
    sink = nc.dram_tensor("sink", [T, HD], F32, kind="ExternalInput")
    w_sl = {
        "q": nc.dram_tensor("wq_sl", [HSL, OQ], F32, kind="ExternalInput"),
        "k": nc.dram_tensor("wk_sl", [HSL, OK], F32, kind="ExternalInput"),
        "v": nc.dram_tensor("wv_sl", [HSL, OV], F32, kind="ExternalInput"),
        "o": nc.dram_tensor("wo_sl", [HSL, OO], F32, kind="ExternalInput"),
    }
    mask_in = nc.dram_tensor("mask", [128, KTILES + TB, T], BF16, kind="ExternalInput")
    # cols 0-3: numel for k,v,q,o ; cols 4-7: 1/numel for k,v,q,o
    wconst = nc.dram_tensor("wconst", [1, 8], F32, kind="ExternalInput")
    out = nc.dram_tensor("out", [T, H], F32, kind="ExternalOutput")

    with tile.TileContext(nc) as tc:
        _build_body(nc, tc, x_sl, cosq, sinq, cosk, sink, w_sl, mask_in, wconst, out)

    nc.compile()
    return nc


def _build_body(nc, tc, x_sl, cosq, sinq, cosk, sink, w_sl, mask_in, wconst, out):
    sync = nc.sync

    with (
        tc.tile_pool(name="dram", bufs=1, space="DRAM") as dram,
        tc.tile_pool(name="const", bufs=1) as constp,
        tc.tile_pool(name="vecs", bufs=1) as vecs,
        tc.tile_pool(name="persist", bufs=1) as persist,
        tc.tile_pool(name="ptrans", bufs=2, space="PSUM") as ptrans,
    ):
        # ---- DRAM bounce buffers for collectives ----
        wag_kv = dram.tile([HSL, OK + OV], BF16)
        wint_kv = dram.tile([H, OK + OV], BF16, addr_space="Shared")
        wag_q = dram.tile([HSL, OQ], BF16)
        wint_q = dram.tile([H, OQ], BF16, addr_space="Shared")
        wag_o = dram.tile([HSL, OO], BF16)
        wint_o = dram.tile([H, OO], BF16, addr_space="Shared")
        ar1_in = dram.tile([1, 8], F32)
        ar1_out = dram.tile([1, 8], F32, addr_space="Shared")
        ar2_in = dram.tile([1, 8], F32)
        ar2_out = dram.tile([1, 8], F32, addr_space="Shared")
        kv_in = dram.tile([128, 4096], BF16)
        kv_out = dram.tile([512, 4096], BF16)

        # ---- constants ----
        ident = constp.tile([128, 128], F32)
        make_identity(nc, ident)
        ones1 = constp.tile([1, 128], F32)
        nc.vector.memset(ones1, 1.0)
        onescol = constp.tile([128, 1], F32)
        nc.vector.memset(onescol, 1.0)
        wconst_sb = constp.tile([1, 8], F32)
        sync.dma_start(wconst_sb, wconst.ap())
        negrnd = constp.tile([128, 1], F32)
        nc.vector.memset(negrnd, -RND)
        epsb = constp.tile([128, 1], F32)
        nc.vector.memset(epsb, EPS)
        cs = {}
        for nm, t in (("cq", cosq), ("sq", sinq), ("ck", cosk), ("sk", sink)):
            c = constp.tile([128, TB, HD], F32, name=f"cs_{nm}")
            sync.dma_start(c, t.ap().rearrange("(a p) d -> p a d", p=128))
            cs[nm] = c
        # persistent activations
        xqT = persist.tile([128, HI_N, T], BF16)  # [h%128, h//128, t]

        # ====== Phase W: weight scales + quantize + pipelined allgathers ======
        def w_scale_group(mats, psmall, ar_in, ar_out, wraws, label):
            """Load slices of `mats`, abs-sum, AllReduce -> [128,4] scale tile
            (cols 0,1 = s_w of mats; cols 2,3 = 1/s_w of mats)."""
            wab = {}
            for m in mats:
                for pt in range(2):
                    wr = wraws[m].tile([128, OW[m]], F32, name=f"wr_{m}{pt}")
                    sync.dma_start(wr, w_sl[m].ap()[pt * 128 : (pt + 1) * 128, :])
                    wab[(m, pt)] = wr
            red0 = vecs.tile([128, 2], F32, name=f"red0_{label}")
            red1 = vecs.tile([128, 2], F32, name=f"red1_{label}")
            for mi, m in enumerate(mats):
                for pt, red in ((0, red0), (1, red1)):
                    nc.vector.tensor_reduce(
                        red[:, mi : mi + 1], wab[(m, pt)], axis=AX, op=OP.add,
                        apply_absolute_value=True,
                    )
            redc = vecs.tile([128, 2], F32, name=f"redc_{label}")
            nc.vector.tensor_add(redc, red0, red1)
            ps = psmall.tile([1, 2], F32, name=f"ps_{label}", tag="psm")
            nc.tensor.matmul(ps, onescol, redc, start=True, stop=True)
            sums = vecs.tile([1, 8], F32, name=f"sums_{label}")
            nc.vector.memset(sums, 0.0)
            nc.scalar.copy(sums[:, 0:2], ps)
            sync.dma_start(ar_in, sums)
            nc.gpsimd.collective_compute(
                "AllReduce", OP.add, replica_groups=[list(range(NC))],
                ins=[ar_in.opt()], outs=[ar_out.opt()],
            )
            g = vecs.tile([1, 8], F32, name=f"g_{label}")
            sync.dma_start(g, ar_out)
            r2 = vecs.tile([1, 2], F32, name=f"r2_{label}")
            nc.vector.reciprocal(r2, g[:, 0:2])
            sw4 = vecs.tile([1, 4], F32, name=f"sw4_{label}")
            ncol = {("k", "v"): (0, 2), ("q", "o"): (2, 4)}[tuple(mats)]
            nc.vector.tensor_mul(sw4[:, 0:2], r2, wconst_sb[:, ncol[0] : ncol[1]])
            nc.vector.tensor_mul(
                sw4[:, 2:4], g[:, 0:2], wconst_sb[:, 4 + ncol[0] : 4 + ncol[1]]
            )
            pb = psmall.tile([128, 4], F32, name=f"pb_{label}", tag="psm")
            nc.tensor.matmul(pb, ones1, sw4, start=True, stop=True)
            sb = vecs.tile([128, 4], F32, name=f"sb_{label}")
            nc.scalar.copy(sb, pb)
            return wab, sb

        def w_quant(wab, m, mi, sb, wtmp, wq8, dst, col0):
            for pt in range(2):
                wr = wab[(m, pt)]
                tmp = wtmp.tile([128, OW[m]], F32, tag="wtmp")
                nc.vector.tensor_scalar(
                    tmp, wr, sb[:, mi : mi + 1], RND, op0=OP.mult, op1=OP.add
                )
                nc.vector.tensor_scalar(
                    tmp, tmp, -RND, 1.0, op0=OP.add, op1=OP.min
                )
                wi = wq8.tile([128, OW[m]], BF16, tag="wi")
                nc.vector.tensor_scalar(wi, tmp, -1.0, None, op0=OP.max)
                sync.dma_start(
                    dst[pt * 128 : (pt + 1) * 128, col0 : col0 + OW[m]], wi
                )

        rswb = {}
        with (
            tc.tile_pool(name="wraw_q", bufs=1) as wraw_q,
            tc.tile_pool(name="wraw_k", bufs=1) as wraw_k,
            tc.tile_pool(name="wraw_v", bufs=1) as wraw_v,
            tc.tile_pool(name="wraw_o", bufs=1) as wraw_o,
            tc.tile_pool(name="wtmp", bufs=2) as wtmp,
            tc.tile_pool(name="wq8", bufs=2) as wq8,
            tc.tile_pool(name="psmall", bufs=2, space="PSUM") as psmall,
        ):
            wraws = {"q": wraw_q, "k": wraw_k, "v": wraw_v, "o": wraw_o}
            wab_kv, sb_kv = w_scale_group(
                ("k", "v"), psmall, ar1_in, ar1_out, wraws, "kv"
            )
            w_quant(wab_kv, "k", 0, sb_kv, wtmp, wq8, wag_kv, 0)
            w_quant(wab_kv, "v", 1, sb_kv, wtmp, wq8, wag_kv, OK)
            nc.gpsimd.collective_compute(
                "AllGather", OP.bypass, replica_groups=[list(range(NC))],
                ins=[wag_kv.opt()], outs=[wint_kv.opt()],
            )
            rswb["k"] = sb_kv[:, 2:3]
            rswb["v"] = sb_kv[:, 3:4]

            wab_qo, sb_qo = w_scale_group(
                ("q", "o"), psmall, ar2_in, ar2_out, wraws, "qo"
            )
            w_quant(wab_qo, "q", 0, sb_qo, wtmp, wq8, wag_q, 0)
            nc.gpsimd.collective_compute(
                "AllGather", OP.bypass, replica_groups=[list(range(NC))],
                ins=[wag_q.opt()], outs=[wint_q.opt()],
            )
            w_quant(wab_qo, "o", 1, sb_qo, wtmp, wq8, wag_o, 0)
            nc.gpsimd.collective_compute(
                "AllGather", OP.bypass, replica_groups=[list(range(NC))],
                ins=[wag_o.opt()], outs=[wint_o.opt()],
            )
            rswb["q"] = sb_qo[:, 2:3]
            rswb["o"] = sb_qo[:, 3:4]

            # ====== Phase X: act_quant(x) + transpose (overlaps Phase W) ======
            dqx = []
            with tc.tile_pool(name="xraw", bufs=2) as xraw:
                for tb in range(TB):
                    xs = xraw.tile([128, H], F32, tag="xs")
                    sync.dma_start(xs, x_sl.ap()[tb * 128 : (tb + 1) * 128, :])
                    axm = vecs.tile([128, 1], F32, name=f"axm{tb}")
                    nc.vector.tensor_reduce(
                        axm, xs, axis=AX, op=OP.max, apply_absolute_value=True
                    )
                    rsx = vecs.tile([128, 1], F32, name=f"rsx{tb}")
                    nc.vector.reciprocal(rsx, axm)
                    sxq = vecs.tile([128, 1], F32, name=f"sxq{tb}")
                    nc.vector.tensor_scalar_mul(sxq, rsx, 127.0)
                    dq = vecs.tile([128, 1], F32, name=f"dqx{tb}")
                    nc.vector.tensor_scalar_mul(dq, axm, 1.0 / 127.0)
                    dqx.append(dq)
                    nc.vector.tensor_scalar(
                        xs, xs, sxq, RND, op0=OP.mult, op1=OP.add
                    )
                    for hg in range(0, HI_N, 4):
                        pt4 = ptrans.tile([128, 4, 128], F32, tag="ptr")
                        for i in range(4):
                            hi = hg + i
                            nc.tensor.transpose(
                                pt4[:, i, :], xs[:, hi * 128 : (hi + 1) * 128], ident
                            )
                        nc.scalar.activation(
                            xqT[:, hg : hg + 4, tb * 128 : (tb + 1) * 128],
                            pt4, AF.Identity, bias=negrnd,
                        )

        # dequant vectors (absmax/127 * 1/s_w)
        dqv = {}
        for m in ("q", "k", "v", "o"):
            for tb in range(TB):
                d = vecs.tile([128, 1], F32, name=f"dqv_{m}{tb}")
                nc.vector.tensor_mul(d, dqx[tb], rswb[m])
                dqv[(m, tb)] = d

        q_sb = persist.tile([128, TB, OQ], F32, tag="qsb")
        k_sb = persist.tile([128, TB, OK], F32)
        v_loc = persist.tile([128, TB, NKV, 130], BF16)
        nc.vector.memset(v_loc, 1.0)
        qT = persist.tile([128, NH, T], BF16)  # [d, head, t]
        kT = persist.tile([128, NKV, T], BF16, tag="t8", padded_shape=[128, HI_N, T])

        def proj(wint_src, col0, o_w, m, dst_fn, wpool, ppool, wchunk=1024,
                 wtag="wst"):
            """dequant(xqT.T @ w_int) over o-chunks; dst_fn(tb, oc) -> out AP
            for the [128, 512] dequantized chunk."""
            src3 = wint_src.rearrange("(hi p) o -> p hi o", p=128)
            nsub = wchunk // 512
            for ocp in range(o_w // wchunk):
                wst = wpool.tile([128, HI_N, wchunk], BF16, tag=wtag)
                sync.dma_start(
                    wst,
                    src3[:, :, col0 + ocp * wchunk : col0 + (ocp + 1) * wchunk],
                )
                for sub in range(nsub):
                    oc = ocp * nsub + sub
                    for tb in range(TB):
                        pp = ppool.tile([128, 512], F32, tag="pp")
                        for hi in range(HI_N):
                            nc.tensor.matmul(
                                pp,
                                xqT[:, hi, tb * 128 : (tb + 1) * 128],
                                wst[:, hi, sub * 512 : (sub + 1) * 512],
                                start=(hi == 0),
                                stop=(hi == HI_N - 1),
                            )
                        nc.vector.tensor_scalar(
                            dst_fn(tb, oc), pp, dqv[(m, tb)], None, op0=OP.mult
                        )

        def rope_batch(src_sb, tb, nh, cosn, sinn, dstT, ropep, label):
            w = nh * 128
            blk = src_sb[:, tb, :]  # [128, w] f32
            sq = ropep.tile([128, w], F32, tag="unf", padded_shape=[128, NH * 128])
            nc.scalar.activation(sq, blk, AF.Square)
            ms = vecs.tile([128, nh], F32, name=f"ms_{label}{tb}")
            nc.vector.tensor_reduce(
                ms, sq.rearrange("p (h d) -> p h d", h=nh), axis=AX, op=OP.add
            )
            rms = vecs.tile([128, nh], F32, name=f"rms_{label}{tb}")
            nc.scalar.activation(rms, ms, AF.Sqrt, scale=1.0 / HD, bias=epsb)
            rn = vecs.tile([128, nh], F32, name=f"rn_{label}{tb}")
            nc.vector.reciprocal(rn, rms)
            rnb = rn.to_broadcast([128, nh, 128])
            blk3 = blk.rearrange("p (h d) -> p h d", h=nh)
            un2 = ropep.tile(
                [128, nh * 128], F32, tag="unf", padded_shape=[128, NH * 128],
                name="un2",
            )
            un = un2.rearrange("p (h d) -> p h d", h=nh)
            nc.vector.tensor_mul(un, blk3, rnb)
            cosb = (
                cs[cosn][:, tb, :]
                .rearrange("p (one d) -> p one d", one=1)
                .to_broadcast([128, nh, 128])
            )
            sinb = (
                cs[sinn][:, tb, :]
                .rearrange("p (one d) -> p one d", one=1)
                .to_broadcast([128, nh, 128])
            )
            ra = ropep.tile([128, nh, 128], F32, tag="ra", padded_shape=[128, NH, 128])
            nc.vector.tensor_mul(ra, un, cosb)
            rb = ropep.tile([128, nh, 128], F32, tag="rb", padded_shape=[128, NH, 128])
            nc.vector.tensor_mul(rb[:, :, 0:64], un[:, :, 64:128], sinb[:, :, 0:64])
            nc.vector.tensor_mul(rb[:, :, 64:128], un[:, :, 0:64], sinb[:, :, 64:128])
            nc.vector.tensor_add(ra, ra, rb)
            for hg in range(0, nh, 4):
                pt4 = ptrans.tile([128, 4, 128], F32, tag="ptr")
                for i in range(4):
                    nc.tensor.transpose(pt4[:, i, :], ra[:, hg + i, :], ident)
                nc.scalar.activation(
                    dstT[:, hg : hg + 4, tb * 128 : (tb + 1) * 128], pt4, AF.Copy
                )

        # ====== K/V projections + K rope + KV allgather ======
        with (
            tc.tile_pool(name="wmm1", bufs=2) as wmm1,
            tc.tile_pool(name="pproj1", bufs=3, space="PSUM") as pproj1,
            tc.tile_pool(name="ropek", bufs=1) as ropek,
        ):
            proj(wint_kv, 0, OK, "k",
                 lambda tb, oc: k_sb[:, tb, oc * 512 : (oc + 1) * 512],
                 wmm1, pproj1)
            proj(wint_kv, OK, OV, "v",
                 lambda tb, oc: v_loc[:, tb, oc * 4 : (oc + 1) * 4, 0:128],
                 wmm1, pproj1)
            for tb in range(TB):
                rope_batch(k_sb, tb, NKV, "ck", "sk", kT, ropek, "k")

            sync.dma_start(
                kv_in[:, 0 : NKV * T].rearrange("p (hk t) -> p hk t", hk=NKV), kT
            )
            sync.dma_start(
                kv_in[:, NKV * T : NKV * T + TB * OV].rearrange(
                    "p (a hk d) -> p a hk d", a=TB, hk=NKV
                ),
                v_loc[:, :, :, 0:128],
            )
            nc.gpsimd.collective_compute(
                "AllGather", OP.bypass,
                replica_groups=[[0, 1, 2, 3], [4, 5, 6, 7]],
                ins=[kv_in.opt()], outs=[kv_out.opt()],
            )

            # ====== Q projection + Q rope (overlap the KV allgather) ======
            proj(wint_q, 0, OQ, "q",
                 lambda tb, oc: q_sb[:, tb, oc * 512 : (oc + 1) * 512],
                 wmm1, pproj1)
            for tb in range(TB):
                rope_batch(q_sb, tb, NH, "cq", "sq", qT, ropek, "q")

        mask_sb = persist.tile([128, KTILES + TB, T], BF16)
        sync.dma_start(mask_sb, mask_in.ap())
        attn = persist.tile([128, TB, H], F32, tag="qsb")  # reuse q_sb slot
        attn_loc = persist.tile([128, TB, NH, 132], F32)

        # ====== attention ======
        with (
            tc.tile_pool(name="pscore", bufs=2, space="PSUM") as pscore,
            tc.tile_pool(name="ppv", bufs=2, space="PSUM") as ppv,
            tc.tile_pool(name="pexp", bufs=2) as pexp,
        ):
            # local part: own K/V tiles (diagonal blocks) - no collective dep
            for h in range(NH):
                hk = h // 2
                pel = pexp.tile([128, TB, T], BF16, tag="pel")
                for a in range(TB):
                    st = pscore.tile([128, T], F32, tag="st")
                    nc.tensor.matmul(
                        st, kT[:, hk, a * 128 : (a + 1) * 128], qT[:, h, :],
                        start=True, stop=True,
                    )
                    nc.scalar.activation(pel[:, a, :], st, AF.Exp, scale=INV_SQRT_HD)
                nc.vector.tensor_mul(pel, pel, mask_sb[:, KTILES : KTILES + TB, :])
                for tb in range(TB):
                    po = ppv.tile([128, 132], F32, tag="po", padded_shape=[128, 132])
                    for a in range(TB):
                        nc.tensor.matmul(
                            po[:, 0:129],
                            pel[:, a, tb * 128 : (tb + 1) * 128],
                            v_loc[:, a, hk, 0:129],
                            start=(a == 0),
                            stop=(a == TB - 1),
                        )
                    nc.vector.tensor_copy(attn_loc[:, tb, h, 0:129], po[:, 0:129])

            # gather readback
            kT_all = persist.tile([128, NKV, KTILES, 128], BF16)
            v_all = persist.tile([128, KTILES, NKV, 130], BF16)
            nc.vector.memset(v_all, 1.0)
            for cb in range(GROUP):
                # kT part: kv_out row = 128*cb + d ; col = hk*256 + a*128 + t
                src_k = kv_out[cb * 128 : (cb + 1) * 128, 0 : NKV * T].rearrange(
                    "d (hk t) -> d hk t", hk=NKV
                )
                sync.dma_start(kT_all[:, :, 2 * cb : 2 * cb + 2, :], src_k)
                # v part: row = 128*cb + p ; col = 2048 + a*1024 + hk*128 + d
                src_v = kv_out[
                    cb * 128 : (cb + 1) * 128, NKV * T : NKV * T + TB * OV
                ].rearrange("p (a hk d) -> p a hk d", a=TB, hk=NKV)
                sync.dma_start(v_all[:, 2 * cb : 2 * cb + 2, :, 0:128], src_v)

            # remote part: strictly-below-diagonal tiles from the allgather
            for h in range(NH):
                hk = h // 2
                pe = pexp.tile([128, KTILES, T], BF16, tag="pe")
                for j in range(KTILES):
                    st = pscore.tile([128, T], F32, tag="st")
                    nc.tensor.matmul(
                        st, kT_all[:, hk, j, :], qT[:, h, :], start=True, stop=True
                    )
                    nc.scalar.activation(
                        pe[:, j, :], st, AF.Exp, scale=INV_SQRT_HD
                    )
                nc.vector.tensor_mul(pe, pe, mask_sb[:, 0:KTILES, :])
                for tb in range(TB):
                    po = ppv.tile([128, 132], F32, tag="po", padded_shape=[128, 132])
                    for j in range(KTILES):
                        nc.tensor.matmul(
                            po[:, 0:129],
                            pe[:, j, tb * 128 : (tb + 1) * 128],
                            v_all[:, j, hk, 0:129],
                            start=(j == 0),
                            stop=(j == KTILES - 1),
                        )
                    cmb = pexp.tile([128, 132], F32, tag="cmb")
                    nc.vector.tensor_add(
                        cmb[:, 0:129], po[:, 0:129], attn_loc[:, tb, h, 0:129]
                    )
                    rden = vecs.tile([128, 1], F32, name=f"rden{h}_{tb}")
                    nc.vector.reciprocal(rden, cmb[:, 128:129])
                    nc.vector.tensor_scalar(
                        attn[:, tb, h * 128 : (h + 1) * 128],
                        cmb[:, 0:128], rden, None, op0=OP.mult,
                    )

        # ====== act_quant(attn) + o_proj ======
        with (
            tc.tile_pool(name="oq", bufs=2) as oq,
            tc.tile_pool(name="wmm2", bufs=3) as wmm2,
            tc.tile_pool(name="pproj2", bufs=3, space="PSUM") as pproj2,
            tc.tile_pool(name="osb", bufs=2) as osb,
        ):
            aT = persist.tile([128, HI_N, T], BF16, tag="t8")
            dqo = []
            for tb in range(TB):
                axm = vecs.tile([128, 1], F32, name=f"oaxm{tb}")
                nc.vector.tensor_reduce(
                    axm, attn[:, tb, :], axis=AX, op=OP.max,
                    apply_absolute_value=True,
                )
                rsx = vecs.tile([128, 1], F32, name=f"orsx{tb}")
                nc.vector.reciprocal(rsx, axm)
                sxq = vecs.tile([128, 1], F32, name=f"osxq{tb}")
                nc.vector.tensor_scalar_mul(sxq, rsx, 127.0)
                dq = vecs.tile([128, 1], F32, name=f"odqx{tb}")
                nc.vector.tensor_scalar_mul(dq, axm, 1.0 / 127.0)
                d2 = vecs.tile([128, 1], F32, name=f"odq2{tb}")
                nc.vector.tensor_mul(d2, dq, rswb["o"])
                dqo.append(d2)
                ar = oq.tile([128, H], F32, tag="ar")
                nc.vector.tensor_scalar(
                    ar, attn[:, tb, :], sxq, RND, op0=OP.mult, op1=OP.add
                )
                for hg in range(0, HI_N, 4):
                    pt4 = ptrans.tile([128, 4, 128], F32, tag="ptr")
                    for i in range(4):
                        hi = hg + i
                        nc.tensor.transpose(
                            pt4[:, i, :], ar[:, hi * 128 : (hi + 1) * 128], ident
                        )
                    nc.scalar.activation(
                        aT[:, hg : hg + 4, tb * 128 : (tb + 1) * 128],
                        pt4, AF.Identity, bias=negrnd,
                    )

            src3 = wint_o.rearrange("(hi p) o -> p hi o", p=128)
            for oc in range(OO // 512):
                wst = wmm2.tile([128, HI_N, 512], BF16, tag="wst2")
                sync.dma_start(wst, src3[:, :, oc * 512 : (oc + 1) * 512])
                for tb in range(TB):
                    pp = pproj2.tile([128, 512], F32, tag="pp2")
                    for hi in range(HI_N):
                        nc.tensor.matmul(
                            pp,
                            aT[:, hi, tb * 128 : (tb + 1) * 128],
                            wst[:, hi, :],
                            start=(hi == 0),
                            stop=(hi == HI_N - 1),
                        )
                    ot = osb.tile([128, 512], F32, tag="ot")
                    nc.vector.tensor_scalar(ot, pp, dqo[tb], None, op0=OP.mult)
                    sync.dma_start(
                        out.ap()[
                            tb * 128 : (tb + 1) * 128, oc * 512 : (oc + 1) * 512
                        ],
                        ot,
                    )


def _host_inputs(x, cos, sin, wq, wk, wv, wo, qn, kn):
    """Build the 8 per-core input maps (pure slicing / layout transforms)."""
    x2 = np.asarray(x, np.float32).reshape(B * S, H)
    cos = np.asarray(cos, np.float32)
    sin = np.asarray(sin, np.float32)
    qn = np.asarray(qn, np.float32)
    kn = np.asarray(kn, np.float32)
    # fold qk-norm weights into rope tables (exact identity when qn=kn=1)
    qn_rot = np.concatenate([qn[HD // 2 :], qn[: HD // 2]])
    kn_rot = np.concatenate([kn[HD // 2 :], kn[: HD // 2]])
    sgn = np.concatenate(
        [-np.ones(HD // 2, np.float32), np.ones(HD // 2, np.float32)]
    )
    cosq_t = cos * qn[None, :]
    sinq_t = sin * (qn_rot * sgn)[None, :]
    cosk_t = cos * kn[None, :]
    sink_t = sin * (kn_rot * sgn)[None, :]

    wt = {
        "q": np.asarray(wq, np.float32).T,  # [H, OQ]
        "k": np.asarray(wk, np.float32).T,
        "v": np.asarray(wv, np.float32).T,
        "o": np.asarray(wo, np.float32).T,  # [H(=in), OO]
    }
    worder = ("k", "v", "q", "o")
    wconst = np.concatenate(
        [
            np.array([WNUMEL[m] for m in worder], np.float32),
            np.array([1.0 / WNUMEL[m] for m in worder], np.float32),
        ]
    ).reshape(1, 8)

    in_maps = []
    for c in range(NC):
        qt = c % GROUP
        t0 = qt * T
        # strict mask [p, j, f]: key (128j+p) fully below this core's window
        p = np.arange(128)[:, None, None]
        j = np.arange(KTILES)[None, :, None]
        f = np.arange(T)[None, None, :]
        strict = ((128 * j + p) < t0) & (f >= 0)
        # diagonal masks for the two local key tiles
        a = np.arange(TB)[None, :, None]
        diag = (128 * a + p) <= f
        mask = np.concatenate([strict, diag], axis=1).astype(ml_dtypes.bfloat16)
        m = {
            "x_sl": np.ascontiguousarray(x2[c * T : (c + 1) * T]),
            "cosq": np.ascontiguousarray(cosq_t[t0 : t0 + T]),
            "sinq": np.ascontiguousarray(sinq_t[t0 : t0 + T]),
            "cosk": np.ascontiguousarray(cosk_t[t0 : t0 + T]),
            "sink": np.ascontiguousarray(sink_t[t0 : t0 + T]),
            "wq_sl": np.ascontiguousarray(wt["q"][c * HSL : (c + 1) * HSL]),
            "wk_sl": np.ascontiguousarray(wt["k"][c * HSL : (c + 1) * HSL]),
            "wv_sl": np.ascontiguousarray(wt["v"][c * HSL : (c + 1) * HSL]),
            "wo_sl": np.ascontiguousarray(wt["o"][c * HSL : (c + 1) * HSL]),
            "mask": mask,
            "wconst": wconst,
        }
        in_maps.append(m)
    return in_maps


def kernel(x, cos, sin, wq, wk, wv, wo, qn, kn):
    if "nc" not in _CACHE:
        _CACHE["nc"] = _build()
    nc = _CACHE["nc"]
    in_maps = _host_inputs(x, cos, sin, wq, wk, wv, wo, qn, kn)
    res = bass_utils.run_bass_kernel_spmd(nc, in_maps, core_ids=list(range(NC)))
    outs = [np.asarray(res.results[c]["out"]) for c in range(NC)]
    return np.concatenate(outs, axis=0).reshape(B, S, H).astype(np.float32)


# revision 24
# speedup vs baseline: 1.4246x; 1.1222x over previous
"""BitNet GQA attention layer on 8 TRN2 NeuronCores.

Sharding: token-parallel. B*S = 2048 tokens -> 256 per core (core c: batch
c//4, quarter c%4). Weights are split 8-way along the contraction dim for
quantization (exact global absmean via tiny AllReduces), then the ternary
integer weights are AllGathered in bf16 (three pipelined AGs: k+v first so
K/V projections start early, then q, then o). K/V are AllGathered within
each batch's 4-core group, hidden under Q-projection + Q-rope. All BitNet
matmuls run as exact integer arithmetic in bf16 (acts in [-128,127],
weights in {-1,0,1}) with fp32 PSUM accumulation; dequantization scales are
applied to the fp32 results.
"""

import sys

sys.path.insert(0, "/opt/trn_rl_repo")

import numpy as np
import ml_dtypes

import concourse.bass as bass
import concourse.mybir as mybir
import concourse.tile as tile
from concourse import bacc
from concourse import bass_utils
from concourse.masks import make_identity

F32 = mybir.dt.float32
BF16 = mybir.dt.bfloat16
FP8 = mybir.dt.float8e4
AX = mybir.AxisListType.X
OP = mybir.AluOpType
AF = mybir.ActivationFunctionType

B, S, H = 2, 1024, 2048
NH, NKV, HD = 16, 8, 128
NC = 8
T = (B * S) // NC  # 256 tokens per core
TB = T // 128  # 2 token tiles per core
HSL = H // NC  # 256 weight rows per core
EPS = 1e-6
RND = 12582912.0  # 1.5 * 2**23: fp32 add => round-to-nearest-even
INV_SQRT_HD = 1.0 / float(np.sqrt(HD))
KTILES = S // 128  # 8 key tiles per batch
GROUP = 4  # cores per batch

OQ, OK, OV, OO = H, NKV * HD, NKV * HD, H  # 2048, 1024, 1024, 2048
OW = {"q": OQ, "k": OK, "v": OV, "o": OO}
WNUMEL = {m: OW[m] * H for m in OW}
HI_N = H // 128  # 16 contraction tiles

_CACHE = {}


def _build():
    nc = bacc.Bacc("TRN2", target_bir_lowering=False, debug=False, num_devices=NC)

    x_sl = nc.dram_tensor("x_sl", [T, H], F32, kind="ExternalInput")
    cosq = nc.dram_tensor("cosq", [T, HD], F32, kind="ExternalInput")
    sinq = nc.dram_tensor("sinq", [T, HD], F32, kind="ExternalInput")
    cosk = nc.dram_tensor("cosk", [T, HD], F32, kind="ExternalInput")
    sink = nc.dram_tensor("sink", [T, HD], F32, kind="ExternalInput")
    w_sl = {
        "q": nc.dram_tensor("wq_sl", [HSL, OQ], F32, kind="ExternalInput"),
        "k": nc.dram_tensor("wk_sl", [HSL, OK], F32, kind="ExternalInput"),
        "v": nc.dram_tensor("wv_sl", [HSL, OV], F32, kind="ExternalInput"),
        "o": nc.dram_tensor("wo_sl", [HSL, OO], F32, kind="ExternalInput"),
    }
    mask_in = nc.dram_tensor("mask", [128, KTILES + TB, T], BF16, kind="ExternalInput")
    # cols 0-3: numel for k,v,q,o ; cols 4-7: 1/numel for k,v,q,o
    wconst = nc.dram_tensor("wconst", [1, 8], F32, kind="ExternalInput")
    out = nc.dram_tensor("out", [T, H], F32, kind="ExternalOutput")

    with tile.TileContext(nc) as tc:
        _build_body(nc, tc, x_sl, cosq, sinq, cosk, sink, w_sl, mask_in, wconst, out)

    nc.compile()
    return nc


def _build_body(nc, tc, x_sl, cosq, sinq, cosk, sink, w_sl, mask_in, wconst, out):
    sync = nc.sync

    with (
        tc.tile_pool(name="dram", bufs=1, space="DRAM") as dram,
        tc.tile_pool(name="const", bufs=1) as constp,
        tc.tile_pool(name="vecs", bufs=1) as vecs,
        tc.tile_pool(name="persist", bufs=1) as persist,
        tc.tile_pool(name="ptrans", bufs=2, space="PSUM") as ptrans,
    ):
        # ---- DRAM bounce buffers for collectives ----
        wag_kv = dram.tile([HSL, OK + OV], FP8)
        wint_kv = dram.tile([H, OK + OV], FP8, addr_space="Shared")
        wag_q = dram.tile([HSL, OQ], FP8)
        wint_q = dram.tile([H, OQ], FP8, addr_space="Shared")
        wag_o = dram.tile([HSL, OO], FP8)
        wint_o = dram.tile([H, OO], FP8, addr_space="Shared")
        ar1_in = dram.tile([1, 8], F32)
        ar1_out = dram.tile([1, 8], F32, addr_space="Shared")
        ar2_in = dram.tile([1, 8], F32)
        ar2_out = dram.tile([1, 8], F32, addr_space="Shared")
        kv_in = dram.tile([128, 4096], BF16)
        kv_out = dram.tile([512, 4096], BF16)

        # ---- constants ----
        ident = constp.tile([128, 128], F32)
        make_identity(nc, ident)
        ones1 = constp.tile([1, 128], F32)
        nc.vector.memset(ones1, 1.0)
        onescol = constp.tile([128, 1], F32)
        nc.vector.memset(onescol, 1.0)
        wconst_sb = constp.tile([1, 8], F32)
        sync.dma_start(wconst_sb, wconst.ap())
        negrnd = constp.tile([128, 1], F32)
        nc.vector.memset(negrnd, -RND)
        epsb = constp.tile([128, 1], F32)
        nc.vector.memset(epsb, EPS)
        cs = {}
        for nm, t in (("cq", cosq), ("sq", sinq), ("ck", cosk), ("sk", sink)):
            c = constp.tile([128, TB, HD], F32, name=f"cs_{nm}")
            sync.dma_start(c, t.ap().rearrange("(a p) d -> p a d", p=128))
            cs[nm] = c
        # persistent activations
        xqT = persist.tile([128, HI_N, T], BF16)  # [h%128, h//128, t]

        # ====== Phase W: weight scales + quantize + pipelined allgathers ======
        def w_scale_group(mats, psmall, ar_in, ar_out, wraws, label):
            """Load slices of `mats`, abs-sum, AllReduce -> [128,4] scale tile
            (cols 0,1 = s_w of mats; cols 2,3 = 1/s_w of mats)."""
            wab = {}
            for m in mats:
                for pt in range(2):
                    wr = wraws[m].tile([128, OW[m]], F32, name=f"wr_{m}{pt}")
                    sync.dma_start(wr, w_sl[m].ap()[pt * 128 : (pt + 1) * 128, :])
                    wab[(m, pt)] = wr
            red0 = vecs.tile([128, 2], F32, name=f"red0_{label}")
            red1 = vecs.tile([128, 2], F32, name=f"red1_{label}")
            for mi, m in enumerate(mats):
                for pt, red in ((0, red0), (1, red1)):
                    nc.vector.tensor_reduce(
                        red[:, mi : mi + 1], wab[(m, pt)], axis=AX, op=OP.add,
                        apply_absolute_value=True,
                    )
            redc = vecs.tile([128, 2], F32, name=f"redc_{label}")
            nc.vector.tensor_add(redc, red0, red1)
            ps = psmall.tile([1, 2], F32, name=f"ps_{label}", tag="psm")
            nc.tensor.matmul(ps, onescol, redc, start=True, stop=True)
            sums = vecs.tile([1, 8], F32, name=f"sums_{label}")
            nc.vector.memset(sums, 0.0)
            nc.scalar.copy(sums[:, 0:2], ps)
            sync.dma_start(ar_in, sums)
            nc.gpsimd.collective_compute(
                "AllReduce", OP.add, replica_groups=[list(range(NC))],
                ins=[ar_in.opt()], outs=[ar_out.opt()],
            )
            g = vecs.tile([1, 8], F32, name=f"g_{label}")
            sync.dma_start(g, ar_out)
            r2 = vecs.tile([1, 2], F32, name=f"r2_{label}")
            nc.vector.reciprocal(r2, g[:, 0:2])
            sw4 = vecs.tile([1, 4], F32, name=f"sw4_{label}")
            ncol = {("k", "v"): (0, 2), ("q", "o"): (2, 4)}[tuple(mats)]
            nc.vector.tensor_mul(sw4[:, 0:2], r2, wconst_sb[:, ncol[0] : ncol[1]])
            nc.vector.tensor_mul(
                sw4[:, 2:4], g[:, 0:2], wconst_sb[:, 4 + ncol[0] : 4 + ncol[1]]
            )
            pb = psmall.tile([128, 4], F32, name=f"pb_{label}", tag="psm")
            nc.tensor.matmul(pb, ones1, sw4, start=True, stop=True)
            sb = vecs.tile([128, 4], F32, name=f"sb_{label}")
            nc.scalar.copy(sb, pb)
            return wab, sb

        def w_quant(wab, m, mi, sb, wtmp, wq8, dst, col0):
            for pt in range(2):
                wr = wab[(m, pt)]
                tmp = wtmp.tile([128, OW[m]], F32, tag="wtmp")
                nc.vector.tensor_scalar(
                    tmp, wr, sb[:, mi : mi + 1], RND, op0=OP.mult, op1=OP.add
                )
                nc.vector.tensor_scalar(
                    tmp, tmp, -RND, 1.0, op0=OP.add, op1=OP.min
                )
                wi = wq8.tile([128, OW[m]], FP8, tag="wi")
                nc.vector.tensor_scalar(wi, tmp, -1.0, None, op0=OP.max)
                sync.dma_start(
                    dst[pt * 128 : (pt + 1) * 128, col0 : col0 + OW[m]], wi
                )

        rswb = {}
        with (
            tc.tile_pool(name="wraw_q", bufs=1) as wraw_q,
            tc.tile_pool(name="wraw_k", bufs=1) as wraw_k,
            tc.tile_pool(name="wraw_v", bufs=1) as wraw_v,
            tc.tile_pool(name="wraw_o", bufs=1) as wraw_o,
            tc.tile_pool(name="wtmp", bufs=2) as wtmp,
            tc.tile_pool(name="wq8", bufs=2) as wq8,
            tc.tile_pool(name="psmall", bufs=2, space="PSUM") as psmall,
        ):
            wraws = {"q": wraw_q, "k": wraw_k, "v": wraw_v, "o": wraw_o}
            wab_kv, sb_kv = w_scale_group(
                ("k", "v"), psmall, ar1_in, ar1_out, wraws, "kv"
            )
            w_quant(wab_kv, "k", 0, sb_kv, wtmp, wq8, wag_kv, 0)
            w_quant(wab_kv, "v", 1, sb_kv, wtmp, wq8, wag_kv, OK)
            nc.gpsimd.collective_compute(
                "AllGather", OP.bypass, replica_groups=[list(range(NC))],
                ins=[wag_kv.opt()], outs=[wint_kv.opt()],
            )
            rswb["k"] = sb_kv[:, 2:3]
            rswb["v"] = sb_kv[:, 3:4]

            wab_qo, sb_qo = w_scale_group(
                ("q", "o"), psmall, ar2_in, ar2_out, wraws, "qo"
            )
            w_quant(wab_qo, "q", 0, sb_qo, wtmp, wq8, wag_q, 0)
            nc.gpsimd.collective_compute(
                "AllGather", OP.bypass, replica_groups=[list(range(NC))],
                ins=[wag_q.opt()], outs=[wint_q.opt()],
            )
            w_quant(wab_qo, "o", 1, sb_qo, wtmp, wq8, wag_o, 0)
            rswb["q"] = sb_qo[:, 2:3]
            rswb["o"] = sb_qo[:, 3:4]

            # ====== Phase X: act_quant(x) + transpose (overlaps Phase W) ======
            dqx = []
            with tc.tile_pool(name="xraw", bufs=2) as xraw:
                for tb in range(TB):
                    xs = xraw.tile([128, H], F32, tag="xs")
                    sync.dma_start(xs, x_sl.ap()[tb * 128 : (tb + 1) * 128, :])
                    axm = vecs.tile([128, 1], F32, name=f"axm{tb}")
                    nc.vector.tensor_reduce(
                        axm, xs, axis=AX, op=OP.max, apply_absolute_value=True
                    )
                    rsx = vecs.tile([128, 1], F32, name=f"rsx{tb}")
                    nc.vector.reciprocal(rsx, axm)
                    sxq = vecs.tile([128, 1], F32, name=f"sxq{tb}")
                    nc.vector.tensor_scalar_mul(sxq, rsx, 127.0)
                    dq = vecs.tile([128, 1], F32, name=f"dqx{tb}")
                    nc.vector.tensor_scalar_mul(dq, axm, 1.0 / 127.0)
                    dqx.append(dq)
                    nc.vector.tensor_scalar(
                        xs, xs, sxq, RND, op0=OP.mult, op1=OP.add
                    )
                    for hg in range(0, HI_N, 4):
                        pt4 = ptrans.tile([128, 4, 128], F32, tag="ptr")
                        for i in range(4):
                            hi = hg + i
                            nc.tensor.transpose(
                                pt4[:, i, :], xs[:, hi * 128 : (hi + 1) * 128], ident
                            )
                        nc.scalar.activation(
                            xqT[:, hg : hg + 4, tb * 128 : (tb + 1) * 128],
                            pt4, AF.Identity, bias=negrnd,
                        )

        # dequant vectors (absmax/127 * 1/s_w)
        dqv = {}
        for m in ("q", "k", "v", "o"):
            for tb in range(TB):
                d = vecs.tile([128, 1], F32, name=f"dqv_{m}{tb}")
                nc.vector.tensor_mul(d, dqx[tb], rswb[m])
                dqv[(m, tb)] = d

        q_sb = persist.tile([128, TB, OQ], F32, tag="qsb")
        k_sb = persist.tile([128, TB, OK], F32)
        v_loc = persist.tile([128, TB, NKV, 130], BF16)
        nc.vector.memset(v_loc, 1.0)
        qT = persist.tile([128, NH, T], BF16)  # [d, head, t]
        kT = persist.tile([128, NKV, T], BF16, tag="t8", padded_shape=[128, HI_N, T])

        def proj(wint_src, col0, o_w, m, dst_fn, wpool, ppool, wchunk=1024,
                 wtag="wst"):
            """dequant(xqT.T @ w_int) over o-chunks; dst_fn(tb, oc) -> out AP
            for the [128, 512] dequantized chunk."""
            src3 = wint_src.rearrange("(hi p) o -> p hi o", p=128)
            nsub = wchunk // 512
            for ocp in range(o_w // wchunk):
                wst = wpool.tile([128, HI_N, wchunk], FP8, tag=wtag)
                sync.dma_start(
                    wst,
                    src3[:, :, col0 + ocp * wchunk : col0 + (ocp + 1) * wchunk],
                )
                for sub in range(nsub):
                    oc = ocp * nsub + sub
                    for tb in range(TB):
                        pp = ppool.tile([128, 512], F32, tag="pp")
                        for hi in range(HI_N):
                            nc.tensor.matmul(
                                pp,
                                xqT[:, hi, tb * 128 : (tb + 1) * 128],
                                wst[:, hi, sub * 512 : (sub + 1) * 512],
                                start=(hi == 0),
                                stop=(hi == HI_N - 1),
                            )
                        nc.vector.tensor_scalar(
                            dst_fn(tb, oc), pp, dqv[(m, tb)], None, op0=OP.mult
                        )

        def rope_batch(src_sb, tb, nh, cosn, sinn, dstT, ropep, label):
            w = nh * 128
            blk = src_sb[:, tb, :]  # [128, w] f32
            sq = ropep.tile([128, w], F32, tag="unf", padded_shape=[128, NH * 128])
            nc.scalar.activation(sq, blk, AF.Square)
            ms = vecs.tile([128, nh], F32, name=f"ms_{label}{tb}")
            nc.vector.tensor_reduce(
                ms, sq.rearrange("p (h d) -> p h d", h=nh), axis=AX, op=OP.add
            )
            rms = vecs.tile([128, nh], F32, name=f"rms_{label}{tb}")
            nc.scalar.activation(rms, ms, AF.Sqrt, scale=1.0 / HD, bias=epsb)
            rn = vecs.tile([128, nh], F32, name=f"rn_{label}{tb}")
            nc.vector.reciprocal(rn, rms)
            rnb = rn.to_broadcast([128, nh, 128])
            blk3 = blk.rearrange("p (h d) -> p h d", h=nh)
            un2 = ropep.tile(
                [128, nh * 128], F32, tag="unf", padded_shape=[128, NH * 128],
                name="un2",
            )
            un = un2.rearrange("p (h d) -> p h d", h=nh)
            nc.vector.tensor_mul(un, blk3, rnb)
            cosb = (
                cs[cosn][:, tb, :]
                .rearrange("p (one d) -> p one d", one=1)
                .to_broadcast([128, nh, 128])
            )
            sinb = (
                cs[sinn][:, tb, :]
                .rearrange("p (one d) -> p one d", one=1)
                .to_broadcast([128, nh, 128])
            )
            ra = ropep.tile([128, nh, 128], F32, tag="ra", padded_shape=[128, NH, 128])
            nc.vector.tensor_mul(ra, un, cosb)
            rb = ropep.tile([128, nh, 128], F32, tag="rb", padded_shape=[128, NH, 128])
            nc.vector.tensor_mul(rb[:, :, 0:64], un[:, :, 64:128], sinb[:, :, 0:64])
            nc.vector.tensor_mul(rb[:, :, 64:128], un[:, :, 0:64], sinb[:, :, 64:128])
            nc.vector.tensor_add(ra, ra, rb)
            for hg in range(0, nh, 4):
                pt4 = ptrans.tile([128, 4, 128], F32, tag="ptr")
                for i in range(4):
                    nc.tensor.transpose(pt4[:, i, :], ra[:, hg + i, :], ident)
                nc.scalar.activation(
                    dstT[:, hg : hg + 4, tb * 128 : (tb + 1) * 128], pt4, AF.Copy
                )

        # ====== K/V projections + K rope + KV allgather ======
        with (
            tc.tile_pool(name="wmm1", bufs=2) as wmm1,
            tc.tile_pool(name="pproj1", bufs=3, space="PSUM") as pproj1,
            tc.tile_pool(name="ropek", bufs=1) as ropek,
        ):
            proj(wint_kv, 0, OK, "k",
                 lambda tb, oc: k_sb[:, tb, oc * 512 : (oc + 1) * 512],
                 wmm1, pproj1)
            proj(wint_kv, OK, OV, "v",
                 lambda tb, oc: v_loc[:, tb, oc * 4 : (oc + 1) * 4, 0:128],
                 wmm1, pproj1)
            for tb in range(TB):
                rope_batch(k_sb, tb, NKV, "ck", "sk", kT, ropek, "k")

            sync.dma_start(
                kv_in[:, 0 : NKV * T].rearrange("p (hk t) -> p hk t", hk=NKV), kT
            )
            sync.dma_start(
                kv_in[:, NKV * T : NKV * T + TB * OV].rearrange(
                    "p (a hk d) -> p a hk d", a=TB, hk=NKV
                ),
                v_loc[:, :, :, 0:128],
            )
            nc.gpsimd.collective_compute(
                "AllGather", OP.bypass,
                replica_groups=[[0, 1, 2, 3], [4, 5, 6, 7]],
                ins=[kv_in.opt()], outs=[kv_out.opt()],
            )
            nc.gpsimd.collective_compute(
                "AllGather", OP.bypass, replica_groups=[list(range(NC))],
                ins=[wag_o.opt()], outs=[wint_o.opt()],
            )

            # ====== Q projection + Q rope (overlap the KV allgather) ======
            proj(wint_q, 0, OQ, "q",
                 lambda tb, oc: q_sb[:, tb, oc * 512 : (oc + 1) * 512],
                 wmm1, pproj1)
            for tb in range(TB):
                rope_batch(q_sb, tb, NH, "cq", "sq", qT, ropek, "q")

        mask_sb = persist.tile([128, KTILES + TB, T], BF16)
        sync.dma_start(mask_sb, mask_in.ap())
        attn = persist.tile([128, TB, H], F32, tag="qsb")  # reuse q_sb slot
        attn_loc = persist.tile([128, TB, NH, 132], F32)

        # ====== attention ======
        with (
            tc.tile_pool(name="pscore", bufs=2, space="PSUM") as pscore,
            tc.tile_pool(name="ppv", bufs=2, space="PSUM") as ppv,
            tc.tile_pool(name="pexp", bufs=2) as pexp,
        ):
            # local part: own K/V tiles (diagonal blocks) - no collective dep
            for h in range(NH):
                hk = h // 2
                pel = pexp.tile([128, TB, T], BF16, tag="pel")
                for a in range(TB):
                    st = pscore.tile([128, T], F32, tag="st")
                    nc.tensor.matmul(
                        st, kT[:, hk, a * 128 : (a + 1) * 128], qT[:, h, :],
                        start=True, stop=True,
                    )
                    nc.scalar.activation(pel[:, a, :], st, AF.Exp, scale=INV_SQRT_HD)
                nc.vector.tensor_mul(pel, pel, mask_sb[:, KTILES : KTILES + TB, :])
                for tb in range(TB):
                    po = ppv.tile([128, 132], F32, tag="po", padded_shape=[128, 132])
                    for a in range(TB):
                        nc.tensor.matmul(
                            po[:, 0:129],
                            pel[:, a, tb * 128 : (tb + 1) * 128],
                            v_loc[:, a, hk, 0:129],
                            start=(a == 0),
                            stop=(a == TB - 1),
                        )
                    nc.vector.tensor_copy(attn_loc[:, tb, h, 0:129], po[:, 0:129])

            # gather readback
            kT_all = persist.tile([128, NKV, KTILES, 128], BF16)
            v_all = persist.tile([128, KTILES, NKV, 130], BF16)
            nc.vector.memset(v_all, 1.0)
            for cb in range(GROUP):
                # kT part: kv_out row = 128*cb + d ; col = hk*256 + a*128 + t
                src_k = kv_out[cb * 128 : (cb + 1) * 128, 0 : NKV * T].rearrange(
                    "d (hk t) -> d hk t", hk=NKV
                )
                sync.dma_start(kT_all[:, :, 2 * cb : 2 * cb + 2, :], src_k)
                # v part: row = 128*cb + p ; col = 2048 + a*1024 + hk*128 + d
                src_v = kv_out[
                    cb * 128 : (cb + 1) * 128, NKV * T : NKV * T + TB * OV
                ].rearrange("p (a hk d) -> p a hk d", a=TB, hk=NKV)
                sync.dma_start(v_all[:, 2 * cb : 2 * cb + 2, :, 0:128], src_v)

            # remote part: strictly-below-diagonal tiles from the allgather
            for h in range(NH):
                hk = h // 2
                pe = pexp.tile([128, KTILES, T], BF16, tag="pe")
                for j in range(KTILES):
                    st = pscore.tile([128, T], F32, tag="st")
                    nc.tensor.matmul(
                        st, kT_all[:, hk, j, :], qT[:, h, :], start=True, stop=True
                    )
                    nc.scalar.activation(
                        pe[:, j, :], st, AF.Exp, scale=INV_SQRT_HD
                    )
                nc.vector.tensor_mul(pe, pe, mask_sb[:, 0:KTILES, :])
                for tb in range(TB):
                    po = ppv.tile([128, 132], F32, tag="po", padded_shape=[128, 132])
                    for j in range(KTILES):
                        nc.tensor.matmul(
                            po[:, 0:129],
                            pe[:, j, tb * 128 : (tb + 1) * 128],
                            v_all[:, j, hk, 0:129],
                            start=(j == 0),
                            stop=(j == KTILES - 1),
                        )
                    cmb = pexp.tile([128, 132], F32, tag="cmb")
                    nc.vector.tensor_add(
                        cmb[:, 0:129], po[:, 0:129], attn_loc[:, tb, h, 0:129]
                    )
                    rden = vecs.tile([128, 1], F32, name=f"rden{h}_{tb}")
                    nc.vector.reciprocal(rden, cmb[:, 128:129])
                    nc.vector.tensor_scalar(
                        attn[:, tb, h * 128 : (h + 1) * 128],
                        cmb[:, 0:128], rden, None, op0=OP.mult,
                    )

        # ====== act_quant(attn) + o_proj ======
        with (
            tc.tile_pool(name="oq", bufs=2) as oq,
            tc.tile_pool(name="wmm2", bufs=3) as wmm2,
            tc.tile_pool(name="pproj2", bufs=3, space="PSUM") as pproj2,
            tc.tile_pool(name="osb", bufs=2) as osb,
        ):
            aT = persist.tile([128, HI_N, T], BF16, tag="t8")
            dqo = []
            for tb in range(TB):
                axm = vecs.tile([128, 1], F32, name=f"oaxm{tb}")
                nc.vector.tensor_reduce(
                    axm, attn[:, tb, :], axis=AX, op=OP.max,
                    apply_absolute_value=True,
                )
                rsx = vecs.tile([128, 1], F32, name=f"orsx{tb}")
                nc.vector.reciprocal(rsx, axm)
                sxq = vecs.tile([128, 1], F32, name=f"osxq{tb}")
                nc.vector.tensor_scalar_mul(sxq, rsx, 127.0)
                dq = vecs.tile([128, 1], F32, name=f"odqx{tb}")
                nc.vector.tensor_scalar_mul(dq, axm, 1.0 / 127.0)
                d2 = vecs.tile([128, 1], F32, name=f"odq2{tb}")
                nc.vector.tensor_mul(d2, dq, rswb["o"])
                dqo.append(d2)
                ar = oq.tile([128, H], F32, tag="ar")
                nc.vector.tensor_scalar(
                    ar, attn[:, tb, :], sxq, RND, op0=OP.mult, op1=OP.add
                )
                for hg in range(0, HI_N, 4):
                    pt4 = ptrans.tile([128, 4, 128], F32, tag="ptr")
                    for i in range(4):
                        hi = hg + i
                        nc.tensor.transpose(
                            pt4[:, i, :], ar[:, hi * 128 : (hi + 1) * 128], ident
                        )
                    nc.scalar.activation(
                        aT[:, hg : hg + 4, tb * 128 : (tb + 1) * 128],
                        pt4, AF.Identity, bias=negrnd,
                    )

            src3 = wint_o.rearrange("(hi p) o -> p hi o", p=128)
            for ocp in range(OO // 1024):
                wst = wmm2.tile([128, HI_N, 1024], FP8, tag="wst2")
                sync.dma_start(wst, src3[:, :, ocp * 1024 : (ocp + 1) * 1024])
                for sub in range(2):
                    oc = ocp * 2 + sub
                    for tb in range(TB):
                        pp = pproj2.tile([128, 512], F32, tag="pp2")
                        for hi in range(HI_N):
                            nc.tensor.matmul(
                                pp,
                                aT[:, hi, tb * 128 : (tb + 1) * 128],
                                wst[:, hi, sub * 512 : (sub + 1) * 512],
                                start=(hi == 0),
                                stop=(hi == HI_N - 1),
                            )
                        ot = osb.tile([128, 512], F32, tag="ot")
                        nc.vector.tensor_scalar(ot, pp, dqo[tb], None, op0=OP.mult)
                        sync.dma_start(
                            out.ap()[
                                tb * 128 : (tb + 1) * 128, oc * 512 : (oc + 1) * 512
                            ],
                            ot,
                        )


def _host_inputs(x, cos, sin, wq, wk, wv, wo, qn, kn):
    """Build the 8 per-core input maps (pure slicing / layout transforms)."""
    x2 = np.asarray(x, np.float32).reshape(B * S, H)
    cos = np.asarray(cos, np.float32)
    sin = np.asarray(sin, np.float32)
    qn = np.asarray(qn, np.float32)
    kn = np.asarray(kn, np.float32)
    # fold qk-norm weights into rope tables (exact identity when qn=kn=1)
    qn_rot = np.concatenate([qn[HD // 2 :], qn[: HD // 2]])
    kn_rot = np.concatenate([kn[HD // 2 :], kn[: HD // 2]])
    sgn = np.concatenate(
        [-np.ones(HD // 2, np.float32), np.ones(HD // 2, np.float32)]
    )
    cosq_t = cos * qn[None, :]
    sinq_t = sin * (qn_rot * sgn)[None, :]
    cosk_t = cos * kn[None, :]
    sink_t = sin * (kn_rot * sgn)[None, :]

    wt = {
        "q": np.asarray(wq, np.float32).T,  # [H, OQ]
        "k": np.asarray(wk, np.float32).T,
        "v": np.asarray(wv, np.float32).T,
        "o": np.asarray(wo, np.float32).T,  # [H(=in), OO]
    }
    worder = ("k", "v", "q", "o")
    wconst = np.concatenate(
        [
            np.array([WNUMEL[m] for m in worder], np.float32),
            np.array([1.0 / WNUMEL[m] for m in worder], np.float32),
        ]
    ).reshape(1, 8)

    in_maps = []
    for c in range(NC):
        qt = c % GROUP
        t0 = qt * T
        # strict mask [p, j, f]: key (128j+p) fully below this core's window
        p = np.arange(128)[:, None, None]
        j = np.arange(KTILES)[None, :, None]
        f = np.arange(T)[None, None, :]
        strict = ((128 * j + p) < t0) & (f >= 0)
        # diagonal masks for the two local key tiles
        a = np.arange(TB)[None, :, None]
        diag = (128 * a + p) <= f
        mask = np.concatenate([strict, diag], axis=1).astype(ml_dtypes.bfloat16)
        m = {
            "x_sl": np.ascontiguousarray(x2[c * T : (c + 1) * T]),
            "cosq": np.ascontiguousarray(cosq_t[t0 : t0 + T]),
            "sinq": np.ascontiguousarray(sinq_t[t0 : t0 + T]),
            "cosk": np.ascontiguousarray(cosk_t[t0 : t0 + T]),
            "sink": np.ascontiguousarray(sink_t[t0 : t0 + T]),
            "wq_sl": np.ascontiguousarray(wt["q"][c * HSL : (c + 1) * HSL]),
            "wk_sl": np.ascontiguousarray(wt["k"][c * HSL : (c + 1) * HSL]),
            "wv_sl": np.ascontiguousarray(wt["v"][c * HSL : (c + 1) * HSL]),
            "wo_sl": np.ascontiguousarray(wt["o"][c * HSL : (c + 1) * HSL]),
            "mask": mask,
            "wconst": wconst,
        }
        in_maps.append(m)
    return in_maps


def kernel(x, cos, sin, wq, wk, wv, wo, qn, kn):
    if "nc" not in _CACHE:
        _CACHE["nc"] = _build()
    nc = _CACHE["nc"]
    in_maps = _host_inputs(x, cos, sin, wq, wk, wv, wo, qn, kn)
    res = bass_utils.run_bass_kernel_spmd(nc, in_maps, core_ids=list(range(NC)))
    outs = [np.asarray(res.results[c]["out"]) for c in range(NC)]
    return np.concatenate(outs, axis=0).reshape(B, S, H).astype(np.float32)
